# revision 19
# baseline (speedup 1.0000x reference)
"""Multi-head attention (B=4, S=2048, D=1024, H=16) on 8 TRN2 NeuronCores.

Sharding (Megatron-style, per spec hint): data-parallel over batch (4) x
tensor-parallel over heads (2 groups of 8). Core c handles batch c//2,
head-group c%2. QKV projections column-sharded, output projection
row-sharded; the two partial bf16 outputs per batch are summed on the host
together with the output bias.

Per-core kernel (one NeuronCore, 8 heads, 2048 tokens):
  - QKV projections run as fp8e4 DoubleRow matmuls (256-deep contraction at
    0.5 cyc/row): weights are pre-scaled x32 (so the lo residual stays out
    of e4m3's subnormal range) and split hi/lo host-side, x likewise; the
    three products w8*x8 + w8*x8l + w8l*x8 give ~9-bit effective precision
    at 0.75x the bf16 instruction cost. The x32^2 score scale is undone in
    the exp; the v-side x32 cancels via the Z column (ones column = 32).
  - Scores transposed ST[k, q]; softmax-exp without max-subtraction, one
    ACT pass per [128, 2, 512] tile -> bf16. A quarter of the exp tiles
    (kt2 odd, second head of each pair) run on the DVE instead as a
    Schraudolph bit-trick (bf16 bits = trunc(A*s + B), max rel err ~3%,
    softmax-cancelled), offloading the ScalarE bottleneck.
  - att@V uses the probabilities as the STATIONARY operand ([128k, 128q]
    slices) and v tiles [128k, 64] as moving, so the output [128q, 64]
    fills all 128 PSUM partitions (half the PE cost of the v-stationary
    form). A head-pair's whole output (4 qt x 2 h x 64) packs into exactly
    one PSUM bank with a single accumulation start/stop; Z accumulates via
    1-column matmuls against the v_aug ones column into a z bank.
  - Normalization is one DVE pass per pair (stride-0 broadcast of 1/Z);
    the normalized [q, feature] tiles go back to feature-major via the DMA
    xbar (dma_start_transpose), not the PE.
  - att@V chains are spliced into the NEXT pair's score loop; k/v/q
    projections and the previous group's output projection are spliced the
    same way (deadline-scheduled), so ScalarE/DVE stream exps with few
    gaps. finish_pair (recip+normalize) is emitted one pair late so the
    in-order DVE queue never parks on unmet deps. The tail pipelines
    per-q-tile: normalize -> xbar transpose -> outproj -> bf16 store.
"""

import sys

if "/opt/trn_rl_repo" not in sys.path:
    sys.path.insert(0, "/opt/trn_rl_repo")

import numpy as np

B, S, D = 4, 2048, 1024
H, DK = 16, 64
NCORES = 8
HC = H // 2            # heads per core
DC = HC * DK           # 512 local features per core
INV_SCALE = 1.0 / 8.0 / (32.0 * 32.0)  # 1/sqrt(DK), /32^2 fp8 weight scale
P = 128
NDCH = D // P          # 8 contraction chunks for projections
NFC = DC // P          # 4 local feature chunks
NKT = S // P           # 16 key tiles
NQG = 4                # query groups
QG = S // NQG          # 512 queries per group
NQT = QG // P          # 4 query tiles per group
VW = DK + 1            # 65: v columns + ones column
NHP = HC // 2          # head pairs

_CACHE = {}


def _build():
    import concourse.bass as bass
    import concourse.bacc as bacc
    import concourse.tile as tile
    import concourse.mybir as mybir
    from concourse.bass import ts, ds

    f32 = mybir.dt.float32
    f32r = mybir.dt.float32r
    bf16 = mybir.dt.bfloat16
    AF = mybir.ActivationFunctionType
    ALU = mybir.AluOpType

    LOG2E = 1.4426950408889634
    SCH_A = INV_SCALE * LOG2E * 128.0
    SCH_B = 16256.0 - 5.5 + 0.5  # centering + trunc->round bias

    nc = bacc.Bacc("TRN2", target_bir_lowering=False, num_devices=NCORES)

    f8 = mybir.dt.float8e4
    DR = mybir.MatmulPerfMode.DoubleRow
    xqT = (nc.dram_tensor("xq8", [D, S], f8, kind="ExternalInput"),
           nc.dram_tensor("xq8l", [D, S], f8, kind="ExternalInput"))
    xkT = (nc.dram_tensor("xk8", [D, S], f8, kind="ExternalInput"),
           nc.dram_tensor("xk8l", [D, S], f8, kind="ExternalInput"))
    xvT = (nc.dram_tensor("xv8", [D, S], f8, kind="ExternalInput"),
           nc.dram_tensor("xv8l", [D, S], f8, kind="ExternalInput"))
    wq = (nc.dram_tensor("wq8", [D, DC], f8, kind="ExternalInput"),
          nc.dram_tensor("wq8l", [D, DC], f8, kind="ExternalInput"))
    wk = (nc.dram_tensor("wk8", [D, DC], f8, kind="ExternalInput"),
          nc.dram_tensor("wk8l", [D, DC], f8, kind="ExternalInput"))
    wv = (nc.dram_tensor("wv8", [D, DC], f8, kind="ExternalInput"),
          nc.dram_tensor("wv8l", [D, DC], f8, kind="ExternalInput"))
    wo = nc.dram_tensor("wo", [DC, D], bf16, kind="ExternalInput")
    bq = nc.dram_tensor("bq", [DC], f32, kind="ExternalInput")
    bk = nc.dram_tensor("bk", [DC], f32, kind="ExternalInput")
    bv = nc.dram_tensor("bv", [DC], f32, kind="ExternalInput")
    out = nc.dram_tensor("out", [S, D], bf16, kind="ExternalOutput")

    with tile.TileContext(nc) as tc:
        with (
            tc.tile_pool(name="persist", bufs=1) as persist,
            tc.tile_pool(name="wts", bufs=2) as wpool,
            tc.tile_pool(name="xin", bufs=4) as xpool,
            tc.tile_pool(name="qt", bufs=2) as qpool,
            tc.tile_pool(name="expst", bufs=18) as epool,
            tc.tile_pool(name="osb", bufs=2) as ospool,
            tc.tile_pool(name="att", bufs=1) as atpool,
            tc.tile_pool(name="small", bufs=2) as spool,
            tc.tile_pool(name="oc", bufs=2) as ocpool,
            tc.tile_pool(name="pp", bufs=2, space="PSUM") as pp,
            tc.tile_pool(name="st", bufs=2, space="PSUM") as st_pool,
            tc.tile_pool(name="av", bufs=1, space="PSUM") as avp,
            tc.tile_pool(name="zp", bufs=1, space="PSUM") as zpool,
        ):
            # ---- persistent SBUF tensors ----
            kT = persist.tile([P, NFC, S], bf16)          # 16KB/part
            v_aug = persist.tile([P, NKT, HC, VW], bf16)  # ~16.6KB/part
            wo_sb = persist.tile([P, NFC, D], bf16)       # 8KB/part
            bq_sb = persist.tile([P, NFC], f32)
            bk_sb = persist.tile([P, NFC], f32)
            bvb = persist.tile([P, DC], f32)              # bias_v broadcast

            nc.sync.dma_start(out=bq_sb, in_=bq.rearrange("(c p) -> p c", p=P))
            nc.sync.dma_start(out=bk_sb, in_=bk.rearrange("(c p) -> p c", p=P))
            bv_ap = bv.ap()
            bvb_src = bass.AP(
                tensor=bv_ap.tensor, offset=bv_ap.offset, ap=[[0, P], *bv_ap.ap]
            )
            nc.sync.dma_start(out=bvb, in_=bvb_src)
            # ones column of v_aug (softmax denominator trick)
            ones_st = persist.tile([P, P], f32)
            nc.vector.memset(ones_st, 32.0)
            nc.vector.tensor_copy(
                out=v_aug[:, :, :, DK],
                in_=ones_st.rearrange("p (k h) -> p k h", k=NKT),
            )

            # ---- emission helpers (PE program order == emission order) ----
            def load_w(w_dram, name, tag="w", bufs=None, fc_split=False,
                       defer=False):
                pair = []
                for i, wd in enumerate(w_dram):
                    pair.append(wpool.tile(
                        [P, NDCH, DC], f8, tag=f"{tag}{i}", name=f"{name}_{i}",
                        bufs=bufs))
                rests = []
                for w_sb, wd in zip(pair, w_dram):
                    wr = wd.rearrange("(c p) f -> p c f", p=P)
                    if fc_split:
                        nc.sync.dma_start(
                            out=w_sb[:, :, 0:DC // 2], in_=wr[:, :, 0:DC // 2])
                        rests.append(lambda w_sb=w_sb, wr=wr: nc.sync.dma_start(
                            out=w_sb[:, :, DC // 2:], in_=wr[:, :, DC // 2:]))
                    else:
                        nc.sync.dma_start(out=w_sb, in_=wr)
                if fc_split:
                    rest = lambda: [r() for r in rests]
                    if defer:
                        return tuple(pair), rest
                    rest()
                return tuple(pair)

            def load_x(xT_dram, g, name, tag="x", bufs=None, split=False):
                pair = []
                for i, xd in enumerate(xT_dram):
                    x_sb = xpool.tile(
                        [P, NDCH, QG], f8, tag=f"{tag}{i}", name=f"{name}_{i}",
                        bufs=bufs)
                    pair.append(x_sb)
                    xr = xd.rearrange("(c p) t -> p c t", p=P)[:, :, ts(g, QG)]
                    if split:
                        h_ = NDCH // 2
                        nc.sync.dma_start(out=x_sb[:, 0:h_, :], in_=xr[:, 0:h_, :])
                        nc.sync.dma_start(out=x_sb[:, h_:, :], in_=xr[:, h_:, :])
                    else:
                        nc.sync.dma_start(out=x_sb, in_=xr)
                return tuple(pair)

            def proj_mms(ps, w_pair, x_pair, fc, half):
                """3-term hi/lo fp8 DoubleRow chain: w8*x8 + w8*x8l + w8l*x8.
                Contraction pairs c of 256 rows; 3 DR matmuls each."""
                w8, w8l = w_pair
                x8, x8l = x_pair
                cs = range(0, NDCH // 4) if half == 0 else (
                    range(NDCH // 4, NDCH // 2) if half == 1
                    else range(NDCH // 2))
                ncp = NDCH // 2
                for c in cs:
                    d = slice(2 * c, 2 * c + 2)
                    for t, (wt, xt) in enumerate(
                        ((w8, x8), (w8, x8l), (w8l, x8))
                    ):
                        nc.tensor.matmul(
                            ps, wt[:, d, ts(fc, P)], xt[:, d, :],
                            start=(c == 0 and t == 0),
                            stop=(c == ncp - 1 and t == 2),
                            perf_mode=DR,
                        )

            def kproj_chain(w_sb, x_sb, g, fc, half=None, state={}):
                if half in (None, 0):
                    state["ps"] = pp.tile(
                        [P, QG], f32, tag="pp", name=f"pk_{g}_{fc}"
                    )
                ps = state["ps"]
                proj_mms(ps, w_sb, x_sb, fc, half)
                if half in (None, 1):
                    nc.vector.tensor_scalar(
                        out=kT[:, fc, ts(g, QG)], in0=ps,
                        scalar1=bk_sb[:, fc : fc + 1], scalar2=None, op0=ALU.add,
                    )

            def qproj_chain(w_sb, x_sb, qT, g, fc, half=None, state={}):
                if half in (None, 0):
                    state["ps"] = pp.tile(
                        [P, QG], f32, tag="pp", name=f"pq_{g}_{fc}"
                    )
                ps = state["ps"]
                proj_mms(ps, w_sb, x_sb, fc, half)
                if half in (None, 1):
                    nc.vector.tensor_scalar(
                        out=qT[:, fc, :], in0=ps,
                        scalar1=bq_sb[:, fc : fc + 1], scalar2=None, op0=ALU.add,
                    )

            def vproj_tile(w_sb, x_sb, kt):
                tt = kt % NQT
                w8, w8l = w_sb
                x8, x8l = x_sb
                ps = pp.tile([P, DC], f32, tag="pp", name=f"pv_{kt}")
                ncp = NDCH // 2
                for c in range(ncp):
                    d = slice(2 * c, 2 * c + 2)
                    for t, (xt, wt) in enumerate(
                        ((x8, w8), (x8, w8l), (x8l, w8))
                    ):
                        nc.tensor.matmul(
                            ps, xt[:, d, ts(tt, P)], wt[:, d, :],
                            start=(c == 0 and t == 0),
                            stop=(c == ncp - 1 and t == 2),
                            perf_mode=DR,
                        )
                nc.vector.tensor_add(
                    out=v_aug[:, kt, :, 0:DK],
                    in0=ps.rearrange("p (h d) -> p h d", h=HC),
                    in1=bvb.rearrange("p (h d) -> p h d", h=HC),
                )

            def outproj_chain(attnT, g, tt, eg, pool=None):
                pool = pool or pp
                ps = pool.tile(
                    [P, DC], f32, tag="pp" if pool is pp else "av",
                    name=f"po_{g}_{tt}_{eg}",
                )
                for fc in range(NFC):
                    nc.tensor.matmul(
                        ps, attnT[:, fc, ts(tt, P)], wo_sb[:, fc, ts(eg, DC)],
                        start=(fc == 0), stop=(fc == NFC - 1),
                    )
                o_sb = ocpool.tile([P, DC], bf16, tag="osb", name=f"ob_{g}_{tt}_{eg}")
                nc.vector.tensor_copy(out=o_sb, in_=ps)
                nc.sync.dma_start(
                    out=out[ds(g * QG + tt * P, P), ts(eg, DC)], in_=o_sb
                )

            # ---- pair state: est tiles + av/z banks, consumed one pair later
            class PairState:
                def __init__(self, g, hp):
                    self.g, self.hp = g, hp
                    self.ests = {}   # h -> list of 8 est tiles [P, 2, QG]
                    self.av = None   # [P, NQT, 2, DK] f32 psum (1 bank)
                    self.zt = None   # [P, QG] f32 psum (1 bank; cols 0:8 used)

            def attv_slice(ps_, s):
                """att@V + Z matmuls consuming est[s] (key tiles 2s, 2s+1)."""
                g, hp = ps_.g, ps_.hp
                if s == 0:
                    ps_.av = avp.tile(
                        [P, NQT, 2, DK], f32, tag="av", name=f"av_{g}_{hp}"
                    )
                    ps_.zt = zpool.tile([P, QG], f32, tag="z", name=f"z_{g}_{hp}")
                last = NKT // 2 - 1
                for kk in range(2):
                    kt = 2 * s + kk
                    for qt in range(NQT):
                        for hh in range(2):
                            h = 2 * hp + hh
                            est = ps_.ests[h][s]
                            stat = est[:, kk, ts(qt, P)]
                            first = s == 0 and kk == 0 and qt == 0 and hh == 0
                            lastm = s == last and kk == 1 and qt == NQT - 1 and hh == 1
                            nc.tensor.matmul(
                                ps_.av[:, qt, hh, :], stat,
                                v_aug[:, kt, h, 0:DK],
                                start=first, stop=lastm,
                            )
                            c = qt * 2 + hh
                            nc.tensor.matmul(
                                ps_.zt[:, c : c + 1], stat,
                                v_aug[:, kt, h, DK:VW],
                                start=first, stop=lastm,
                            )

            def finish_pair(ps_, o_sb_tiles, qts=None):
                """reciprocal + normalize for a finished pair.

                qts: restrict the normalize to these q-tiles (tail
                pipelining); reciprocal runs only when qts is None or
                starts at qt 0."""
                g, hp = ps_.g, ps_.hp
                if qts is None or qts[0] == 0:
                    ps_.rz = spool.tile(
                        [P, NQT, 2], f32r, tag="rz", name=f"rz_{g}_{hp}"
                    )
                    with nc.allow_low_precision("softmax denom reciprocal"):
                        nc.vector.reciprocal(
                            out=ps_.rz,
                            in_=ps_.zt[:, 0 : 2 * NQT].rearrange(
                                "p (q h) -> p q h", q=NQT
                            ),
                        )
                o_sb = o_sb_tiles[g]
                if qts is None:
                    nc.vector.tensor_tensor(
                        out=o_sb[:, :, 2 * hp : 2 * hp + 2, :],
                        in0=ps_.av,
                        in1=ps_.rz.unsqueeze(-1).broadcast_to([P, NQT, 2, DK]),
                        op=ALU.mult,
                    )
                else:
                    for qt in qts:
                        nc.vector.tensor_tensor(
                            out=o_sb[:, qt, 2 * hp : 2 * hp + 2, :],
                            in0=ps_.av[:, qt, :, :],
                            in1=ps_.rz[:, qt, :].unsqueeze(-1).broadcast_to(
                                [P, 2, DK]),
                            op=ALU.mult,
                        )

            def transposes(g, o_sb_tiles, attnT):
                o_sb = o_sb_tiles[g]
                for qt in range(NQT):
                    for fc in range(NFC):
                        nc.sync.dma_start_transpose(
                            out=attnT[:, fc, ts(qt, P)],
                            in_=o_sb[:, qt, 2 * fc : 2 * fc + 2, :],
                        )

            # =========== prelude ===========
            # DMA order tuned so the first-score chain (wk fc01, xk0, wq
            # fc01, xq0) clears in ~10us and fill-phase consumers (xv0, wv,
            # xk1-3) arrive before their spliced chains need them.
            wk_sb, wk_rest = load_w(wk, "w_k", fc_split=True, defer=True)
            xk_sbs = [load_x(xkT, 0, "x_k_0", tag="xk", bufs=4, split=True)]
            wq_sb, wq_rest = load_w(wq, "w_q", tag="wq", bufs=1, fc_split=True,
                                    defer=True)
            xq_tiles = {0: load_x(xqT, 0, "x_q_0", tag="xq", bufs=2, split=True)}
            kproj_chain(wk_sb, xk_sbs[0], 0, 0, half=0)
            kproj_chain(wk_sb, xk_sbs[0], 0, 0, half=1)

            qst = {0: qpool.tile([P, NFC, QG], bf16, tag="qT", name="qT_0")}
            qproj_chain(wq_sb, xq_tiles[0], qst[0], 0, 0, half=0)
            qproj_chain(wq_sb, xq_tiles[0], qst[0], 0, 0, half=1)

            xk_sbs.append(load_x(xkT, 1, "x_k_1", tag="xk", bufs=4))
            wv_sb = load_w(wv, "w_v")
            xv_tiles = {0: load_x(xvT, 0, "x_v_0", tag="xv", bufs=2)}
            xk_sbs.append(load_x(xkT, 2, "x_k_2", tag="xk", bufs=4))
            xk_sbs.append(load_x(xkT, 3, "x_k_3", tag="xk", bufs=4))
            wk_rest()
            wq_rest()
            nc.sync.dma_start(out=wo_sb, in_=wo.rearrange("(c p) e -> p c e", p=P))

            # =========== splice schedule ===========
            # pair index p = 4*g + hp runs score loop slots 0..7; sched[p][s]
            # is a list of thunks emitted before slot s's score matmuls.
            sched = {p: {s: [] for s in range(8)} for p in range(16)}

            def at(p, s, fn):
                sched[p][s].append(fn)

            # kproj: fc=0 for kg>=1 early in pair 0; fc=f in pair f-1... but
            # pair (0,hp) reads kT chunk hp for all kt: chunk fc must be fully
            # projected (all 4 kg) before pair (0,fc) starts.
            for kg, s_ in [(1, 0), (2, 1), (3, 3)]:
                at(0, s_, lambda kg=kg: kproj_chain(wk_sb, xk_sbs[kg], kg, 0))
            for fc in range(1, 4):
                for kg in range(4):
                    at(fc - 1, 2 * kg + 1, lambda kg=kg, fc=fc: kproj_chain(
                        wk_sb, xk_sbs[kg], kg, fc))
            # vproj: 10 tiles in pair 0 (extra on later slots), 6 in pair 1;
            # v_aug[kt] needed by attV(0,0) slice s=kt//2 at pair 1 slot s.
            # xv loads run >=2 slots ahead of their first vproj consumer.
            for vg, (p_, s_) in {1: (0, 1), 2: (0, 5), 3: (0, 7)}.items():
                at(p_, s_, lambda vg=vg: xv_tiles.__setitem__(
                    vg, load_x(xvT, vg, f"x_v_{vg}", tag="xv", bufs=2)))
            vq = [(0, 0, 1), (0, 1, 1), (0, 2, 1), (0, 3, 1), (0, 4, 2),
                  (0, 5, 2), (0, 6, 2), (0, 7, 2), (1, 0, 2), (1, 1, 2),
                  (1, 2, 2)]
            kt_next = 0
            for p_, s_, n_ in vq:
                for _ in range(n_):
                    if kt_next >= NKT:
                        break
                    kt = kt_next
                    kt_next += 1
                    at(p_, s_, lambda kt=kt: vproj_tile(
                        wv_sb, xv_tiles[kt // NQT], kt))
            # qproj for pair p+1 at pair p slot 5 (+ xq loads 2 pairs early)
            for p in range(15):
                g1, fc1 = divmod(p + 1, 4)
                if fc1 == 0 and g1 > 0:
                    at(p - 2 if p >= 2 else 0, 1, lambda g1=g1: xq_tiles.__setitem__(
                        g1, load_x(xqT, g1, f"x_q_{g1}", tag="xq", bufs=2)))
                    at(p, 5, lambda g1=g1: (
                        qst.__setitem__(g1, qpool.tile(
                            [P, NFC, QG], bf16, tag="qT", name=f"qT_{g1}")),
                        qproj_chain(wq_sb, xq_tiles[g1], qst[g1], g1, 0,
                                    half=0))[-1])
                    at(p, 7, lambda g1=g1: qproj_chain(
                        wq_sb, xq_tiles[g1], qst[g1], g1, 0, half=1))
                else:
                    at(p, 5, lambda g1=g1, fc1=fc1: qproj_chain(
                        wq_sb, xq_tiles[g1], qst[g1], g1, fc1, half=0))
                    at(p, 7, lambda g1=g1, fc1=fc1: qproj_chain(
                        wq_sb, xq_tiles[g1], qst[g1], g1, fc1, half=1))
            # outproj(g) chains spliced into pairs of group g+1
            op_slots = [(1, 4), (1, 6), (2, 2), (2, 4), (2, 6), (3, 2),
                        (3, 4), (3, 6)]
            attnT_holder = {}
            for g in range(3):
                for i, (hp_, s_) in enumerate(op_slots):
                    tt, eg = divmod(i, 2)
                    at(4 * (g + 1) + hp_, s_, lambda g=g, tt=tt, eg=eg: outproj_chain(
                        attnT_holder[g], g, tt, eg))

            # =========== main loop ===========
            o_sb_tiles = {}
            prev_pair = None   # PairState consumed by current pair's splices
            done_pair = None   # PairState whose attV completed last pair
            # (its finish_pair runs at the START of this pair so the DVE
            # queue never parks on unmet deps — DVE is in-order)

            for p in range(16):
                g, hp = divmod(p, 4)
                if g not in o_sb_tiles:
                    o_sb_tiles[g] = ospool.tile(
                        [P, NQT, HC, DK], bf16, tag="osb2", name=f"o_{g}"
                    )
                cur = PairState(g, hp)
                qT = qst[g]
                for kt2 in range(NKT // 2):
                    if kt2 == 0 and done_pair is not None:
                        finish_pair(done_pair, o_sb_tiles)
                        if done_pair.hp == NHP - 1:
                            gg = done_pair.g
                            attnT_holder[gg] = atpool.tile(
                                [P, NFC, QG], bf16, tag="attnT", name=f"aT_{gg}"
                            )
                            transposes(gg, o_sb_tiles, attnT_holder[gg])
                        done_pair = None
                    def emit_splices():
                        if prev_pair is not None:
                            attv_slice(prev_pair, kt2)
                        for fn in sched[p][kt2]:
                            fn()

                    def emit_scores():
                        sts = {}
                        for hh in range(2):
                            h = 2 * hp + hh
                            sts[h] = st_pool.tile(
                                [P, 2, QG], f32, tag="st",
                                name=f"st_{g}_{h}_{kt2}"
                            )
                        for kk in range(2):
                            kt = 2 * kt2 + kk
                            for hh in range(2):
                                h = 2 * hp + hh
                                r0 = hh * DK
                                nc.tensor.matmul(
                                    sts[h][:, kk, :],
                                    kT[r0 : r0 + DK, hp, ts(kt, P)],
                                    qT[r0 : r0 + DK, hp, :],
                                    start=True, stop=True,
                                    tile_position=(r0, 0),
                                )
                        for hh in range(2):
                            h = 2 * hp + hh
                            e = epool.tile(
                                [P, 2, QG], bf16, tag="est",
                                name=f"est_{g}_{h}_{kt2}"
                            )
                            cur.ests.setdefault(h, []).append(e)
                            if hh == 1 and kt2 % 2 == 1:
                                # Schraudolph exp on DVE: bf16 bit pattern of
                                # exp(s*INV_SCALE) ~= trunc(A*s + B); offloads
                                # 25% of the exp stream from ScalarE (max rel
                                # err ~3%, partially cancelled by softmax)
                                nc.vector.tensor_scalar(
                                    out=e.bitcast(mybir.dt.int16),
                                    in0=sts[h],
                                    scalar1=SCH_A, scalar2=SCH_B,
                                    op0=ALU.mult, op1=ALU.add,
                                )
                            else:
                                nc.scalar.activation(
                                    out=e, in_=sts[h], func=AF.Exp,
                                    scale=INV_SCALE
                                )

                    # fill phase (pairs 0-2): ACT is starved, so feed it
                    # scores before the heavy projection splices; steady
                    # state: splices first (PE uses the st-ring wait time)
                    if p < 3:
                        emit_scores()
                        emit_splices()
                    else:
                        emit_splices()
                        emit_scores()
                # previous pair's attV is complete; finish it at the start
                # of the next pair (deps met there, no DVE queue parking)
                done_pair = prev_pair
                prev_pair = cur

            # =========== tail: last pair's attV + outproj of group 3 ====
            # per-qt pipelining: as soon as qt's normalize lands, its
            # transposes, outproj chains and output DMA flow while the PE
            # works the next qt.
            finish_pair(done_pair, o_sb_tiles)
            for s in range(NKT // 2):
                attv_slice(prev_pair, s)
            attnT_holder[3] = atpool.tile(
                [P, NFC, QG], bf16, tag="attnT", name="aT_3"
            )
            o_sb3 = o_sb_tiles[3]
            for qt in range(NQT):
                finish_pair(prev_pair, o_sb_tiles, qts=[qt])
                for fc in range(NFC):
                    nc.sync.dma_start_transpose(
                        out=attnT_holder[3][:, fc, ts(qt, P)],
                        in_=o_sb3[:, qt, 2 * fc : 2 * fc + 2, :],
                    )
                for eg in range(2):
                    outproj_chain(attnT_holder[3], 3, qt, eg)

    nc.compile()
    return nc


def _get_nc(debug=False):
    if "nc" not in _CACHE:
        _CACHE["nc"] = _build()
    return _CACHE["nc"]


def _tf32(a):
    """Round fp32 to the TF32 grid (10-bit mantissa, round-to-nearest-even)."""
    u = np.ascontiguousarray(a, dtype=np.float32).view(np.uint32)
    u = (u + np.uint32(0xFFF) + ((u >> np.uint32(13)) & np.uint32(1))) & np.uint32(
        0xFFFFE000
    )
    return u.view(np.float32)


def _bf16(a):
    import ml_dtypes

    return np.ascontiguousarray(a, dtype=np.float32).astype(ml_dtypes.bfloat16)


def _make_in_maps(inputs):
    q = np.asarray(inputs["query"], dtype=np.float32)
    k = np.asarray(inputs["key"], dtype=np.float32)
    v = np.asarray(inputs["value"], dtype=np.float32)
    wq = np.asarray(inputs["wq"], dtype=np.float32)
    wk = np.asarray(inputs["wk"], dtype=np.float32)
    wv = np.asarray(inputs["wv"], dtype=np.float32)
    wo = np.asarray(inputs["wo"], dtype=np.float32)
    bq = np.asarray(inputs["bq"], dtype=np.float32)
    bk = np.asarray(inputs["bk"], dtype=np.float32)
    bv = np.asarray(inputs["bv"], dtype=np.float32)

    import ml_dtypes

    def _hl(a):
        hi = np.ascontiguousarray(a, dtype=np.float32).astype(
            ml_dtypes.float8_e4m3)
        lo = (a - hi.astype(np.float32)).astype(ml_dtypes.float8_e4m3)
        return hi, lo

    WS = 32.0  # fp8 weight pre-scale (undone via exp scale / ones column)
    xT = [(_hl(q[b].T), _hl(k[b].T), _hl(v[b].T)) for b in range(B)]
    in_maps = []
    for c in range(NCORES):
        b, g = divmod(c, 2)
        sl = slice(g * DC, (g + 1) * DC)
        wq8, wq8l = _hl(wq[:, sl] * WS)
        wk8, wk8l = _hl(wk[:, sl] * WS)
        wv8, wv8l = _hl(wv[:, sl] * WS)
        in_maps.append(
            {
                "xq8": xT[b][0][0], "xq8l": xT[b][0][1],
                "xk8": xT[b][1][0], "xk8l": xT[b][1][1],
                "xv8": xT[b][2][0], "xv8l": xT[b][2][1],
                "wq8": wq8, "wq8l": wq8l,
                "wk8": wk8, "wk8l": wk8l,
                "wv8": wv8, "wv8l": wv8l,
                "wo": _bf16(wo[sl, :]),
                "bq": np.ascontiguousarray(bq[sl] * WS),
                "bk": np.ascontiguousarray(bk[sl] * WS),
                "bv": np.ascontiguousarray(bv[sl] * WS),
            }
        )
    return in_maps


def run(inputs, **kwargs):
    """Run the kernel; returns (full_output, BassKernelResults)."""
    from concourse.bass_utils import run_bass_kernel_spmd

    kwargs.pop("debug", None)
    nc = _get_nc()
    in_maps = _make_in_maps(inputs)
    res = run_bass_kernel_spmd(nc, in_maps, core_ids=list(range(NCORES)), **kwargs)
    bo = np.asarray(inputs["bo"], dtype=np.float32)
    final = np.empty((B, S, D), np.float32)
    for b in range(B):
        final[b] = (
            res.results[2 * b]["out"].astype(np.float32)
            + res.results[2 * b + 1]["out"].astype(np.float32)
            + bo
        )
    return final, res


def kernel(**inputs):
    return run(inputs)[0]


# revision 21
# speedup vs baseline: 1.0001x; 1.0001x over previous
"""Multi-head attention (B=4, S=2048, D=1024, H=16) on 8 TRN2 NeuronCores.

Sharding (Megatron-style, per spec hint): data-parallel over batch (4) x
tensor-parallel over heads (2 groups of 8). Core c handles batch c//2,
head-group c%2. QKV projections column-sharded, output projection
row-sharded; the two partial bf16 outputs per batch are summed on the host
together with the output bias.

Per-core kernel (one NeuronCore, 8 heads, 2048 tokens):
  - QKV projections run as fp8e4 DoubleRow matmuls (256-deep contraction at
    0.5 cyc/row): weights are pre-scaled x32 (so the lo residual stays out
    of e4m3's subnormal range) and split hi/lo host-side, x likewise; the
    three products w8*x8 + w8*x8l + w8l*x8 give ~9-bit effective precision
    at 0.75x the bf16 instruction cost. The x32^2 score scale is undone in
    the exp; the v-side x32 cancels via the Z column (ones column = 32).
  - Scores transposed ST[k, q]; softmax-exp without max-subtraction, one
    ACT pass per [128, 2, 512] tile -> bf16. A quarter of the exp tiles
    (kt2 odd, second head of each pair) run on the DVE instead as a
    Schraudolph bit-trick (bf16 bits = trunc(A*s + B), max rel err ~3%,
    softmax-cancelled), offloading the ScalarE bottleneck.
  - att@V uses the probabilities as the STATIONARY operand ([128k, 128q]
    slices) and v tiles [128k, 64] as moving, so the output [128q, 64]
    fills all 128 PSUM partitions (half the PE cost of the v-stationary
    form). A head-pair's whole output (4 qt x 2 h x 64) packs into exactly
    one PSUM bank with a single accumulation start/stop; Z accumulates via
    1-column matmuls against the v_aug ones column into a z bank.
  - Normalization is one DVE pass per pair (stride-0 broadcast of 1/Z);
    the normalized [q, feature] tiles go back to feature-major via the DMA
    xbar (dma_start_transpose), not the PE.
  - att@V chains are spliced into the NEXT pair's score loop; k/v/q
    projections and the previous group's output projection are spliced the
    same way (deadline-scheduled), so ScalarE/DVE stream exps with few
    gaps. finish_pair (recip+normalize) is emitted one pair late so the
    in-order DVE queue never parks on unmet deps. The tail pipelines
    per-q-tile: normalize -> xbar transpose -> outproj -> bf16 store.
"""

import sys

if "/opt/trn_rl_repo" not in sys.path:
    sys.path.insert(0, "/opt/trn_rl_repo")

import numpy as np

B, S, D = 4, 2048, 1024
H, DK = 16, 64
NCORES = 8
HC = H // 2            # heads per core
DC = HC * DK           # 512 local features per core
INV_SCALE = 1.0 / 8.0 / (32.0 * 32.0)  # 1/sqrt(DK), /32^2 fp8 weight scale
P = 128
NDCH = D // P          # 8 contraction chunks for projections
NFC = DC // P          # 4 local feature chunks
NKT = S // P           # 16 key tiles
NQG = 4                # query groups
QG = S // NQG          # 512 queries per group
NQT = QG // P          # 4 query tiles per group
VW = DK + 1            # 65: v columns + ones column
NHP = HC // 2          # head pairs

_CACHE = {}


def _build():
    import concourse.bass as bass
    import concourse.bacc as bacc
    import concourse.tile as tile
    import concourse.mybir as mybir
    from concourse.bass import ts, ds

    f32 = mybir.dt.float32
    f32r = mybir.dt.float32r
    bf16 = mybir.dt.bfloat16
    AF = mybir.ActivationFunctionType
    ALU = mybir.AluOpType

    LOG2E = 1.4426950408889634
    SCH_A = INV_SCALE * LOG2E * 128.0
    SCH_B = 16256.0 - 5.5 + 0.5  # centering + trunc->round bias

    nc = bacc.Bacc("TRN2", target_bir_lowering=False, num_devices=NCORES)

    f8 = mybir.dt.float8e4
    DR = mybir.MatmulPerfMode.DoubleRow
    xqT = (nc.dram_tensor("xq8", [D, S], f8, kind="ExternalInput"),
           nc.dram_tensor("xq8l", [D, S], f8, kind="ExternalInput"))
    xkT = (nc.dram_tensor("xk8", [D, S], f8, kind="ExternalInput"),
           nc.dram_tensor("xk8l", [D, S], f8, kind="ExternalInput"))
    xvT = (nc.dram_tensor("xv8", [D, S], f8, kind="ExternalInput"),
           nc.dram_tensor("xv8l", [D, S], f8, kind="ExternalInput"))
    wq = (nc.dram_tensor("wq8", [D, DC], f8, kind="ExternalInput"),
          nc.dram_tensor("wq8l", [D, DC], f8, kind="ExternalInput"))
    wk = (nc.dram_tensor("wk8", [D, DC], f8, kind="ExternalInput"),
          nc.dram_tensor("wk8l", [D, DC], f8, kind="ExternalInput"))
    wv = (nc.dram_tensor("wv8", [D, DC], f8, kind="ExternalInput"),
          nc.dram_tensor("wv8l", [D, DC], f8, kind="ExternalInput"))
    wo = nc.dram_tensor("wo", [DC, D], bf16, kind="ExternalInput")
    bq = nc.dram_tensor("bq", [DC], f32, kind="ExternalInput")
    bk = nc.dram_tensor("bk", [DC], f32, kind="ExternalInput")
    bv = nc.dram_tensor("bv", [DC], f32, kind="ExternalInput")
    out = nc.dram_tensor("out", [S, D], bf16, kind="ExternalOutput")

    with tile.TileContext(nc) as tc:
        with (
            tc.tile_pool(name="persist", bufs=1) as persist,
            tc.tile_pool(name="wts", bufs=2) as wpool,
            tc.tile_pool(name="xin", bufs=4) as xpool,
            tc.tile_pool(name="qt", bufs=2) as qpool,
            tc.tile_pool(name="expst", bufs=18) as epool,
            tc.tile_pool(name="osb", bufs=2) as ospool,
            tc.tile_pool(name="att", bufs=1) as atpool,
            tc.tile_pool(name="small", bufs=2) as spool,
            tc.tile_pool(name="oc", bufs=2) as ocpool,
            tc.tile_pool(name="pp", bufs=2, space="PSUM") as pp,
            tc.tile_pool(name="st", bufs=2, space="PSUM") as st_pool,
            tc.tile_pool(name="av", bufs=1, space="PSUM") as avp,
            tc.tile_pool(name="zp", bufs=1, space="PSUM") as zpool,
        ):
            # ---- persistent SBUF tensors ----
            kT = persist.tile([P, NFC, S], bf16)          # 16KB/part
            v_aug = persist.tile([P, NKT, HC, VW], bf16)  # ~16.6KB/part
            wo_sb = persist.tile([P, NFC, D], bf16)       # 8KB/part
            bq_sb = persist.tile([P, NFC], f32)
            bk_sb = persist.tile([P, NFC], f32)
            bvb = persist.tile([P, DC], f32)              # bias_v broadcast

            nc.sync.dma_start(out=bq_sb, in_=bq.rearrange("(c p) -> p c", p=P))
            nc.sync.dma_start(out=bk_sb, in_=bk.rearrange("(c p) -> p c", p=P))
            bv_ap = bv.ap()
            bvb_src = bass.AP(
                tensor=bv_ap.tensor, offset=bv_ap.offset, ap=[[0, P], *bv_ap.ap]
            )
            nc.sync.dma_start(out=bvb, in_=bvb_src)
            # ones column of v_aug (softmax denominator trick)
            ones_st = persist.tile([P, P], f32)
            nc.vector.memset(ones_st, 32.0)
            nc.vector.tensor_copy(
                out=v_aug[:, :, :, DK],
                in_=ones_st.rearrange("p (k h) -> p k h", k=NKT),
            )

            # ---- emission helpers (PE program order == emission order) ----
            def load_w(w_dram, name, tag="w", bufs=None, fc_split=False,
                       defer=False):
                pair = []
                for i, wd in enumerate(w_dram):
                    pair.append(wpool.tile(
                        [P, NDCH, DC], f8, tag=f"{tag}{i}", name=f"{name}_{i}",
                        bufs=bufs))
                rests = []
                for w_sb, wd in zip(pair, w_dram):
                    wr = wd.rearrange("(c p) f -> p c f", p=P)
                    if fc_split:
                        nc.sync.dma_start(
                            out=w_sb[:, :, 0:DC // 2], in_=wr[:, :, 0:DC // 2])
                        rests.append(lambda w_sb=w_sb, wr=wr: nc.sync.dma_start(
                            out=w_sb[:, :, DC // 2:], in_=wr[:, :, DC // 2:]))
                    else:
                        nc.sync.dma_start(out=w_sb, in_=wr)
                if fc_split:
                    rest = lambda: [r() for r in rests]
                    if defer:
                        return tuple(pair), rest
                    rest()
                return tuple(pair)

            def load_x(xT_dram, g, name, tag="x", bufs=None, split=False):
                pair = []
                srcs = []
                for i, xd in enumerate(xT_dram):
                    x_sb = xpool.tile(
                        [P, NDCH, QG], f8, tag=f"{tag}{i}", name=f"{name}_{i}",
                        bufs=bufs)
                    pair.append(x_sb)
                    srcs.append(
                        xd.rearrange("(c p) t -> p c t", p=P)[:, :, ts(g, QG)])
                if split:
                    h_ = NDCH // 2
                    for dsl in (slice(0, h_), slice(h_, NDCH)):
                        for x_sb, xr in zip(pair, srcs):
                            nc.sync.dma_start(
                                out=x_sb[:, dsl, :], in_=xr[:, dsl, :])
                else:
                    for x_sb, xr in zip(pair, srcs):
                        nc.sync.dma_start(out=x_sb, in_=xr)
                return tuple(pair)

            def proj_mms(ps, w_pair, x_pair, fc, half):
                """3-term hi/lo fp8 DoubleRow chain: w8*x8 + w8*x8l + w8l*x8.
                Contraction pairs c of 256 rows; 3 DR matmuls each."""
                w8, w8l = w_pair
                x8, x8l = x_pair
                cs = range(0, NDCH // 4) if half == 0 else (
                    range(NDCH // 4, NDCH // 2) if half == 1
                    else range(NDCH // 2))
                ncp = NDCH // 2
                for c in cs:
                    d = slice(2 * c, 2 * c + 2)
                    for t, (wt, xt) in enumerate(
                        ((w8, x8), (w8, x8l), (w8l, x8))
                    ):
                        nc.tensor.matmul(
                            ps, wt[:, d, ts(fc, P)], xt[:, d, :],
                            start=(c == 0 and t == 0),
                            stop=(c == ncp - 1 and t == 2),
                            perf_mode=DR,
                        )

            def kproj_chain(w_sb, x_sb, g, fc, half=None, state={}):
                if half in (None, 0):
                    state["ps"] = pp.tile(
                        [P, QG], f32, tag="pp", name=f"pk_{g}_{fc}"
                    )
                ps = state["ps"]
                proj_mms(ps, w_sb, x_sb, fc, half)
                if half in (None, 1):
                    nc.vector.tensor_scalar(
                        out=kT[:, fc, ts(g, QG)], in0=ps,
                        scalar1=bk_sb[:, fc : fc + 1], scalar2=None, op0=ALU.add,
                    )

            def qproj_chain(w_sb, x_sb, qT, g, fc, half=None, state={}):
                if half in (None, 0):
                    state["ps"] = pp.tile(
                        [P, QG], f32, tag="pp", name=f"pq_{g}_{fc}"
                    )
                ps = state["ps"]
                proj_mms(ps, w_sb, x_sb, fc, half)
                if half in (None, 1):
                    nc.vector.tensor_scalar(
                        out=qT[:, fc, :], in0=ps,
                        scalar1=bq_sb[:, fc : fc + 1], scalar2=None, op0=ALU.add,
                    )

            def vproj_tile(w_sb, x_sb, kt):
                tt = kt % NQT
                w8, w8l = w_sb
                x8, x8l = x_sb
                ps = pp.tile([P, DC], f32, tag="pp", name=f"pv_{kt}")
                ncp = NDCH // 2
                for c in range(ncp):
                    d = slice(2 * c, 2 * c + 2)
                    for t, (xt, wt) in enumerate(
                        ((x8, w8), (x8, w8l), (x8l, w8))
                    ):
                        nc.tensor.matmul(
                            ps, xt[:, d, ts(tt, P)], wt[:, d, :],
                            start=(c == 0 and t == 0),
                            stop=(c == ncp - 1 and t == 2),
                            perf_mode=DR,
                        )
                nc.vector.tensor_add(
                    out=v_aug[:, kt, :, 0:DK],
                    in0=ps.rearrange("p (h d) -> p h d", h=HC),
                    in1=bvb.rearrange("p (h d) -> p h d", h=HC),
                )

            def outproj_chain(attnT, g, tt, eg, pool=None, copy_act=False):
                pool = pool or pp
                ps = pool.tile(
                    [P, DC], f32, tag="pp" if pool is pp else "av",
                    name=f"po_{g}_{tt}_{eg}",
                )
                for fc in range(NFC):
                    nc.tensor.matmul(
                        ps, attnT[:, fc, ts(tt, P)], wo_sb[:, fc, ts(eg, DC)],
                        start=(fc == 0), stop=(fc == NFC - 1),
                    )
                o_sb = ocpool.tile([P, DC], bf16, tag="osb", name=f"ob_{g}_{tt}_{eg}")
                if copy_act:
                    nc.scalar.copy(out=o_sb, in_=ps)
                else:
                    nc.vector.tensor_copy(out=o_sb, in_=ps)
                nc.sync.dma_start(
                    out=out[ds(g * QG + tt * P, P), ts(eg, DC)], in_=o_sb
                )

            # ---- pair state: est tiles + av/z banks, consumed one pair later
            class PairState:
                def __init__(self, g, hp):
                    self.g, self.hp = g, hp
                    self.ests = {}   # h -> list of 8 est tiles [P, 2, QG]
                    self.av = None   # [P, NQT, 2, DK] f32 psum (1 bank)
                    self.zt = None   # [P, QG] f32 psum (1 bank; cols 0:8 used)

            def attv_slice(ps_, s):
                """att@V + Z matmuls consuming est[s] (key tiles 2s, 2s+1)."""
                g, hp = ps_.g, ps_.hp
                if s == 0:
                    ps_.av = avp.tile(
                        [P, NQT, 2, DK], f32, tag="av", name=f"av_{g}_{hp}"
                    )
                    ps_.zt = zpool.tile([P, QG], f32, tag="z", name=f"z_{g}_{hp}")
                last = NKT // 2 - 1
                for kk in range(2):
                    kt = 2 * s + kk
                    for qt in range(NQT):
                        for hh in range(2):
                            h = 2 * hp + hh
                            est = ps_.ests[h][s]
                            stat = est[:, kk, ts(qt, P)]
                            first = s == 0 and kk == 0 and qt == 0 and hh == 0
                            lastm = s == last and kk == 1 and qt == NQT - 1 and hh == 1
                            nc.tensor.matmul(
                                ps_.av[:, qt, hh, :], stat,
                                v_aug[:, kt, h, 0:DK],
                                start=first, stop=lastm,
                            )
                            c = qt * 2 + hh
                            nc.tensor.matmul(
                                ps_.zt[:, c : c + 1], stat,
                                v_aug[:, kt, h, DK:VW],
                                start=first, stop=lastm,
                            )

            def finish_pair(ps_, o_sb_tiles, qts=None):
                """reciprocal + normalize for a finished pair.

                qts: restrict the normalize to these q-tiles (tail
                pipelining); reciprocal runs only when qts is None or
                starts at qt 0."""
                g, hp = ps_.g, ps_.hp
                if qts is None or qts[0] == 0:
                    ps_.rz = spool.tile(
                        [P, NQT, 2], f32r, tag="rz", name=f"rz_{g}_{hp}"
                    )
                    with nc.allow_low_precision("softmax denom reciprocal"):
                        nc.vector.reciprocal(
                            out=ps_.rz,
                            in_=ps_.zt[:, 0 : 2 * NQT].rearrange(
                                "p (q h) -> p q h", q=NQT
                            ),
                        )
                o_sb = o_sb_tiles[g]
                if qts is None:
                    nc.vector.tensor_tensor(
                        out=o_sb[:, :, 2 * hp : 2 * hp + 2, :],
                        in0=ps_.av,
                        in1=ps_.rz.unsqueeze(-1).broadcast_to([P, NQT, 2, DK]),
                        op=ALU.mult,
                    )
                else:
                    for qt in qts:
                        nc.vector.tensor_tensor(
                            out=o_sb[:, qt, 2 * hp : 2 * hp + 2, :],
                            in0=ps_.av[:, qt, :, :],
                            in1=ps_.rz[:, qt, :].unsqueeze(-1).broadcast_to(
                                [P, 2, DK]),
                            op=ALU.mult,
                        )

            def transposes(g, o_sb_tiles, attnT):
                o_sb = o_sb_tiles[g]
                for qt in range(NQT):
                    for fc in range(NFC):
                        nc.sync.dma_start_transpose(
                            out=attnT[:, fc, ts(qt, P)],
                            in_=o_sb[:, qt, 2 * fc : 2 * fc + 2, :],
                        )

            # =========== prelude ===========
            # DMA order tuned so the first-score chain (wk fc01, xk0, wq
            # fc01, xq0) clears in ~10us and fill-phase consumers (xv0, wv,
            # xk1-3) arrive before their spliced chains need them.
            wk_sb, wk_rest = load_w(wk, "w_k", fc_split=True, defer=True)
            xk_sbs = [load_x(xkT, 0, "x_k_0", tag="xk", bufs=4, split=True)]
            wq_sb, wq_rest = load_w(wq, "w_q", tag="wq", bufs=1, fc_split=True,
                                    defer=True)
            xq_tiles = {0: load_x(xqT, 0, "x_q_0", tag="xq", bufs=2, split=True)}
            kproj_chain(wk_sb, xk_sbs[0], 0, 0, half=0)
            kproj_chain(wk_sb, xk_sbs[0], 0, 0, half=1)

            qst = {0: qpool.tile([P, NFC, QG], bf16, tag="qT", name="qT_0")}
            qproj_chain(wq_sb, xq_tiles[0], qst[0], 0, 0, half=0)
            qproj_chain(wq_sb, xq_tiles[0], qst[0], 0, 0, half=1)

            xk_sbs.append(load_x(xkT, 1, "x_k_1", tag="xk", bufs=4))
            wv_sb = load_w(wv, "w_v")
            xv_tiles = {0: load_x(xvT, 0, "x_v_0", tag="xv", bufs=2)}
            xk_sbs.append(load_x(xkT, 2, "x_k_2", tag="xk", bufs=4))
            xk_sbs.append(load_x(xkT, 3, "x_k_3", tag="xk", bufs=4))
            wk_rest()
            wq_rest()
            nc.sync.dma_start(out=wo_sb, in_=wo.rearrange("(c p) e -> p c e", p=P))

            # =========== splice schedule ===========
            # pair index p = 4*g + hp runs score loop slots 0..7; sched[p][s]
            # is a list of thunks emitted before slot s's score matmuls.
            sched = {p: {s: [] for s in range(8)} for p in range(16)}

            def at(p, s, fn):
                sched[p][s].append(fn)

            # kproj: fc=0 for kg>=1 early in pair 0; fc=f in pair f-1... but
            # pair (0,hp) reads kT chunk hp for all kt: chunk fc must be fully
            # projected (all 4 kg) before pair (0,fc) starts.
            for kg, s_ in [(1, 0), (2, 1), (3, 3)]:
                at(0, s_, lambda kg=kg: kproj_chain(wk_sb, xk_sbs[kg], kg, 0))
            for fc in range(1, 4):
                for kg in range(4):
                    at(fc - 1, 2 * kg + 1, lambda kg=kg, fc=fc: kproj_chain(
                        wk_sb, xk_sbs[kg], kg, fc))
            # vproj: 10 tiles in pair 0 (extra on later slots), 6 in pair 1;
            # v_aug[kt] needed by attV(0,0) slice s=kt//2 at pair 1 slot s.
            # xv loads run >=2 slots ahead of their first vproj consumer.
            for vg, (p_, s_) in {1: (0, 1), 2: (0, 5), 3: (0, 7)}.items():
                at(p_, s_, lambda vg=vg: xv_tiles.__setitem__(
                    vg, load_x(xvT, vg, f"x_v_{vg}", tag="xv", bufs=2)))
            vq = [(0, 0, 1), (0, 1, 1), (0, 2, 1), (0, 3, 1), (0, 4, 2),
                  (0, 5, 2), (0, 6, 2), (0, 7, 2), (1, 0, 2), (1, 1, 2),
                  (1, 2, 2)]
            kt_next = 0
            for p_, s_, n_ in vq:
                for _ in range(n_):
                    if kt_next >= NKT:
                        break
                    kt = kt_next
                    kt_next += 1
                    at(p_, s_, lambda kt=kt: vproj_tile(
                        wv_sb, xv_tiles[kt // NQT], kt))
            # qproj for pair p+1 at pair p slot 5 (+ xq loads 2 pairs early)
            for p in range(15):
                g1, fc1 = divmod(p + 1, 4)
                if fc1 == 0 and g1 > 0:
                    at(p - 2 if p >= 2 else 0, 1, lambda g1=g1: xq_tiles.__setitem__(
                        g1, load_x(xqT, g1, f"x_q_{g1}", tag="xq", bufs=2)))
                    at(p, 5, lambda g1=g1: (
                        qst.__setitem__(g1, qpool.tile(
                            [P, NFC, QG], bf16, tag="qT", name=f"qT_{g1}")),
                        qproj_chain(wq_sb, xq_tiles[g1], qst[g1], g1, 0,
                                    half=0))[-1])
                    at(p, 7, lambda g1=g1: qproj_chain(
                        wq_sb, xq_tiles[g1], qst[g1], g1, 0, half=1))
                else:
                    at(p, 5, lambda g1=g1, fc1=fc1: qproj_chain(
                        wq_sb, xq_tiles[g1], qst[g1], g1, fc1, half=0))
                    at(p, 7, lambda g1=g1, fc1=fc1: qproj_chain(
                        wq_sb, xq_tiles[g1], qst[g1], g1, fc1, half=1))
            # outproj(g) chains spliced into pairs of group g+1
            op_slots = [(1, 4), (1, 6), (2, 2), (2, 4), (2, 6), (3, 2),
                        (3, 4), (3, 6)]
            attnT_holder = {}
            for g in range(3):
                for i, (hp_, s_) in enumerate(op_slots):
                    tt, eg = divmod(i, 2)
                    at(4 * (g + 1) + hp_, s_, lambda g=g, tt=tt, eg=eg: outproj_chain(
                        attnT_holder[g], g, tt, eg))

            # =========== main loop ===========
            o_sb_tiles = {}
            prev_pair = None   # PairState consumed by current pair's splices
            done_pair = None   # PairState whose attV completed last pair
            # (its finish_pair runs at the START of this pair so the DVE
            # queue never parks on unmet deps — DVE is in-order)

            for p in range(16):
                g, hp = divmod(p, 4)
                if g not in o_sb_tiles:
                    o_sb_tiles[g] = ospool.tile(
                        [P, NQT, HC, DK], bf16, tag="osb2", name=f"o_{g}"
                    )
                cur = PairState(g, hp)
                qT = qst[g]
                for kt2 in range(NKT // 2):
                    if kt2 == 0 and done_pair is not None:
                        finish_pair(done_pair, o_sb_tiles)
                        if done_pair.hp == NHP - 1:
                            gg = done_pair.g
                            attnT_holder[gg] = atpool.tile(
                                [P, NFC, QG], bf16, tag="attnT", name=f"aT_{gg}"
                            )
                            transposes(gg, o_sb_tiles, attnT_holder[gg])
                        done_pair = None
                    def emit_splices():
                        if prev_pair is not None:
                            attv_slice(prev_pair, kt2)
                        for fn in sched[p][kt2]:
                            fn()

                    def emit_scores():
                        sts = {}
                        for hh in range(2):
                            h = 2 * hp + hh
                            sts[h] = st_pool.tile(
                                [P, 2, QG], f32, tag="st",
                                name=f"st_{g}_{h}_{kt2}"
                            )
                        for kk in range(2):
                            kt = 2 * kt2 + kk
                            for hh in range(2):
                                h = 2 * hp + hh
                                r0 = hh * DK
                                nc.tensor.matmul(
                                    sts[h][:, kk, :],
                                    kT[r0 : r0 + DK, hp, ts(kt, P)],
                                    qT[r0 : r0 + DK, hp, :],
                                    start=True, stop=True,
                                    tile_position=(r0, 0),
                                )
                        for hh in range(2):
                            h = 2 * hp + hh
                            e = epool.tile(
                                [P, 2, QG], bf16, tag="est",
                                name=f"est_{g}_{h}_{kt2}"
                            )
                            cur.ests.setdefault(h, []).append(e)
                            if hh == 1 and kt2 % 2 == 1:
                                # Schraudolph exp on DVE: bf16 bit pattern of
                                # exp(s*INV_SCALE) ~= trunc(A*s + B); offloads
                                # 25% of the exp stream from ScalarE (max rel
                                # err ~3%, partially cancelled by softmax)
                                nc.vector.tensor_scalar(
                                    out=e.bitcast(mybir.dt.int16),
                                    in0=sts[h],
                                    scalar1=SCH_A, scalar2=SCH_B,
                                    op0=ALU.mult, op1=ALU.add,
                                )
                            else:
                                nc.scalar.activation(
                                    out=e, in_=sts[h], func=AF.Exp,
                                    scale=INV_SCALE
                                )

                    # fill phase (pairs 0-2): ACT is starved, so feed it
                    # scores before the heavy projection splices; steady
                    # state: splices first (PE uses the st-ring wait time)
                    if p < 3:
                        emit_scores()
                        emit_splices()
                    else:
                        emit_splices()
                        emit_scores()
                # previous pair's attV is complete; finish it at the start
                # of the next pair (deps met there, no DVE queue parking)
                done_pair = prev_pair
                prev_pair = cur

            # =========== tail: last pair's attV + outproj of group 3 ====
            # per-qt pipelining: as soon as qt's normalize lands, its
            # transposes, outproj chains and output DMA flow while the PE
            # works the next qt.
            finish_pair(done_pair, o_sb_tiles)
            for s in range(NKT // 2):
                attv_slice(prev_pair, s)
            attnT_holder[3] = atpool.tile(
                [P, NFC, QG], bf16, tag="attnT", name="aT_3"
            )
            o_sb3 = o_sb_tiles[3]
            for qt in range(NQT):
                finish_pair(prev_pair, o_sb_tiles, qts=[qt])
                for fc in range(NFC):
                    nc.sync.dma_start_transpose(
                        out=attnT_holder[3][:, fc, ts(qt, P)],
                        in_=o_sb3[:, qt, 2 * fc : 2 * fc + 2, :],
                    )
                for eg in range(2):
                    outproj_chain(attnT_holder[3], 3, qt, eg, copy_act=True)

    nc.compile()
    return nc


def _get_nc(debug=False):
    if "nc" not in _CACHE:
        _CACHE["nc"] = _build()
    return _CACHE["nc"]


def _tf32(a):
    """Round fp32 to the TF32 grid (10-bit mantissa, round-to-nearest-even)."""
    u = np.ascontiguousarray(a, dtype=np.float32).view(np.uint32)
    u = (u + np.uint32(0xFFF) + ((u >> np.uint32(13)) & np.uint32(1))) & np.uint32(
        0xFFFFE000
    )
    return u.view(np.float32)


def _bf16(a):
    import ml_dtypes

    return np.ascontiguousarray(a, dtype=np.float32).astype(ml_dtypes.bfloat16)


def _make_in_maps(inputs):
    q = np.asarray(inputs["query"], dtype=np.float32)
    k = np.asarray(inputs["key"], dtype=np.float32)
    v = np.asarray(inputs["value"], dtype=np.float32)
    wq = np.asarray(inputs["wq"], dtype=np.float32)
    wk = np.asarray(inputs["wk"], dtype=np.float32)
    wv = np.asarray(inputs["wv"], dtype=np.float32)
    wo = np.asarray(inputs["wo"], dtype=np.float32)
    bq = np.asarray(inputs["bq"], dtype=np.float32)
    bk = np.asarray(inputs["bk"], dtype=np.float32)
    bv = np.asarray(inputs["bv"], dtype=np.float32)

    import ml_dtypes

    def _hl(a):
        hi = np.ascontiguousarray(a, dtype=np.float32).astype(
            ml_dtypes.float8_e4m3)
        lo = (a - hi.astype(np.float32)).astype(ml_dtypes.float8_e4m3)
        return hi, lo

    WS = 32.0  # fp8 weight pre-scale (undone via exp scale / ones column)
    xT = [(_hl(q[b].T), _hl(k[b].T), _hl(v[b].T)) for b in range(B)]
    in_maps = []
    for c in range(NCORES):
        b, g = divmod(c, 2)
        sl = slice(g * DC, (g + 1) * DC)
        wq8, wq8l = _hl(wq[:, sl] * WS)
        wk8, wk8l = _hl(wk[:, sl] * WS)
        wv8, wv8l = _hl(wv[:, sl] * WS)
        in_maps.append(
            {
                "xq8": xT[b][0][0], "xq8l": xT[b][0][1],
                "xk8": xT[b][1][0], "xk8l": xT[b][1][1],
                "xv8": xT[b][2][0], "xv8l": xT[b][2][1],
                "wq8": wq8, "wq8l": wq8l,
                "wk8": wk8, "wk8l": wk8l,
                "wv8": wv8, "wv8l": wv8l,
                "wo": _bf16(wo[sl, :]),
                "bq": np.ascontiguousarray(bq[sl] * WS),
                "bk": np.ascontiguousarray(bk[sl] * WS),
                "bv": np.ascontiguousarray(bv[sl] * WS),
            }
        )
    return in_maps


def run(inputs, **kwargs):
    """Run the kernel; returns (full_output, BassKernelResults)."""
    from concourse.bass_utils import run_bass_kernel_spmd

    kwargs.pop("debug", None)
    nc = _get_nc()
    in_maps = _make_in_maps(inputs)
    res = run_bass_kernel_spmd(nc, in_maps, core_ids=list(range(NCORES)), **kwargs)
    bo = np.asarray(inputs["bo"], dtype=np.float32)
    final = np.empty((B, S, D), np.float32)
    for b in range(B):
        final[b] = (
            res.results[2 * b]["out"].astype(np.float32)
            + res.results[2 * b + 1]["out"].astype(np.float32)
            + bo
        )
    return final, res


def kernel(**inputs):
    return run(inputs)[0]


# revision 23
# speedup vs baseline: 1.0106x; 1.0106x over previous
"""Multi-head attention (B=4, S=2048, D=1024, H=16) on 8 TRN2 NeuronCores.

Sharding (Megatron-style, per spec hint): data-parallel over batch (4) x
tensor-parallel over heads (2 groups of 8). Core c handles batch c//2,
head-group c%2. QKV projections column-sharded, output projection
row-sharded; the two partial bf16 outputs per batch are summed on the host
together with the output bias.

Per-core kernel (one NeuronCore, 8 heads, 2048 tokens):
  - QKV projections run as fp8e4 DoubleRow matmuls (256-deep contraction at
    0.5 cyc/row): weights are pre-scaled x32 (so the lo residual stays out
    of e4m3's subnormal range) and split hi/lo host-side, x likewise; the
    three products w8*x8 + w8*x8l + w8l*x8 give ~9-bit effective precision
    at 0.75x the bf16 instruction cost. The x32^2 score scale is undone in
    the exp; the v-side x32 cancels via the Z column (ones column = 32).
  - Scores transposed ST[k, q]; softmax-exp without max-subtraction, one
    ACT pass per [128, 2, 512] tile -> bf16. A quarter of the exp tiles
    (kt2 odd, second head of each pair) run on the DVE instead as a
    Schraudolph bit-trick (bf16 bits = trunc(A*s + B), max rel err ~3%,
    softmax-cancelled), offloading the ScalarE bottleneck.
  - att@V uses the probabilities as the STATIONARY operand ([128k, 128q]
    slices) and v tiles [128k, 64] as moving, so the output [128q, 64]
    fills all 128 PSUM partitions (half the PE cost of the v-stationary
    form). A head-pair's whole output (4 qt x 2 h x 64) packs into exactly
    one PSUM bank with a single accumulation start/stop; Z accumulates via
    1-column matmuls against the v_aug ones column into a z bank.
  - Normalization is one DVE pass per pair (stride-0 broadcast of 1/Z);
    the normalized [q, feature] tiles go back to feature-major via the DMA
    xbar (dma_start_transpose), not the PE.
  - att@V chains are spliced into the NEXT pair's score loop; k/v/q
    projections and the previous group's output projection are spliced the
    same way (deadline-scheduled), so ScalarE/DVE stream exps with few
    gaps. finish_pair (recip+normalize) is emitted one pair late so the
    in-order DVE queue never parks on unmet deps. The tail pipelines
    per-q-tile: normalize -> xbar transpose -> outproj -> bf16 store.
"""

import sys

if "/opt/trn_rl_repo" not in sys.path:
    sys.path.insert(0, "/opt/trn_rl_repo")

import numpy as np

B, S, D = 4, 2048, 1024
H, DK = 16, 64
NCORES = 8
HC = H // 2            # heads per core
DC = HC * DK           # 512 local features per core
INV_SCALE = 1.0 / 8.0 / (32.0 * 32.0)  # 1/sqrt(DK), /32^2 fp8 weight scale
P = 128
NDCH = D // P          # 8 contraction chunks for projections
NFC = DC // P          # 4 local feature chunks
NKT = S // P           # 16 key tiles
NQG = 4                # query groups
QG = S // NQG          # 512 queries per group
NQT = QG // P          # 4 query tiles per group
VW = DK + 1            # 65: v columns + ones column
NHP = HC // 2          # head pairs

_CACHE = {}


def _build():
    import concourse.bass as bass
    import concourse.bacc as bacc
    import concourse.tile as tile
    import concourse.mybir as mybir
    from concourse.bass import ts, ds

    f32 = mybir.dt.float32
    f32r = mybir.dt.float32r
    bf16 = mybir.dt.bfloat16
    AF = mybir.ActivationFunctionType
    ALU = mybir.AluOpType

    LOG2E = 1.4426950408889634
    SCH_A = INV_SCALE * LOG2E * 128.0
    SCH_B = 16256.0 - 5.5 + 0.5  # centering + trunc->round bias

    nc = bacc.Bacc("TRN2", target_bir_lowering=False, num_devices=NCORES)

    f8 = mybir.dt.float8e4
    DR = mybir.MatmulPerfMode.DoubleRow
    xqT = (nc.dram_tensor("xq8", [D, S], f8, kind="ExternalInput"),
           nc.dram_tensor("xq8l", [D, S], f8, kind="ExternalInput"))
    xkT = (nc.dram_tensor("xk8", [D, S], f8, kind="ExternalInput"),
           nc.dram_tensor("xk8l", [D, S], f8, kind="ExternalInput"))
    xvT = (nc.dram_tensor("xv8", [D, S], f8, kind="ExternalInput"),
           nc.dram_tensor("xv8l", [D, S], f8, kind="ExternalInput"))
    wq = (nc.dram_tensor("wq8", [D, DC], f8, kind="ExternalInput"),
          nc.dram_tensor("wq8l", [D, DC], f8, kind="ExternalInput"))
    wk = (nc.dram_tensor("wk8", [D, DC], f8, kind="ExternalInput"),
          nc.dram_tensor("wk8l", [D, DC], f8, kind="ExternalInput"))
    wv = (nc.dram_tensor("wv8", [D, DC], f8, kind="ExternalInput"),
          nc.dram_tensor("wv8l", [D, DC], f8, kind="ExternalInput"))
    wo = nc.dram_tensor("wo", [DC, D], bf16, kind="ExternalInput")
    bq = nc.dram_tensor("bq", [DC], f32, kind="ExternalInput")
    bk = nc.dram_tensor("bk", [DC], f32, kind="ExternalInput")
    bv = nc.dram_tensor("bv", [DC], f32, kind="ExternalInput")
    out = nc.dram_tensor("out", [S, D], bf16, kind="ExternalOutput")

    with tile.TileContext(nc) as tc:
        with (
            tc.tile_pool(name="persist", bufs=1) as persist,
            tc.tile_pool(name="wts", bufs=2) as wpool,
            tc.tile_pool(name="xin", bufs=4) as xpool,
            tc.tile_pool(name="qt", bufs=2) as qpool,
            tc.tile_pool(name="expst", bufs=18) as epool,
            tc.tile_pool(name="osb", bufs=2) as ospool,
            tc.tile_pool(name="att", bufs=1) as atpool,
            tc.tile_pool(name="small", bufs=2) as spool,
            tc.tile_pool(name="oc", bufs=2) as ocpool,
            tc.tile_pool(name="pp", bufs=2, space="PSUM") as pp,
            tc.tile_pool(name="st", bufs=2, space="PSUM") as st_pool,
            tc.tile_pool(name="av", bufs=1, space="PSUM") as avp,
            tc.tile_pool(name="zp", bufs=1, space="PSUM") as zpool,
        ):
            # ---- persistent SBUF tensors ----
            kT = persist.tile([P, NFC, S], bf16)          # 16KB/part
            v_aug = persist.tile([P, NKT, HC, VW], bf16)  # ~16.6KB/part
            wo_sb = persist.tile([P, NFC, D], bf16)       # 8KB/part
            bq_sb = persist.tile([P, NFC], f32)
            bk_sb = persist.tile([P, NFC], f32)
            bvb = persist.tile([P, DC], f32)              # bias_v broadcast

            def small_loads():
                nc.sync.dma_start(
                    out=bq_sb, in_=bq.rearrange("(c p) -> p c", p=P))
                nc.sync.dma_start(
                    out=bk_sb, in_=bk.rearrange("(c p) -> p c", p=P))
                bv_ap = bv.ap()
                bvb_src = bass.AP(
                    tensor=bv_ap.tensor, offset=bv_ap.offset,
                    ap=[[0, P], *bv_ap.ap]
                )
                nc.sync.dma_start(out=bvb, in_=bvb_src)
            # ones column of v_aug (softmax denominator trick)
            ones_st = persist.tile([P, P], f32)
            nc.vector.memset(ones_st, 32.0)
            nc.vector.tensor_copy(
                out=v_aug[:, :, :, DK],
                in_=ones_st.rearrange("p (k h) -> p k h", k=NKT),
            )

            # ---- emission helpers (PE program order == emission order) ----
            def load_w(w_dram, name, tag="w", bufs=None, fc_split=False,
                       defer=False):
                pair = []
                for i, wd in enumerate(w_dram):
                    pair.append(wpool.tile(
                        [P, NDCH, DC], f8, tag=f"{tag}{i}", name=f"{name}_{i}",
                        bufs=bufs))
                rests = []
                for w_sb, wd in zip(pair, w_dram):
                    wr = wd.rearrange("(c p) f -> p c f", p=P)
                    if fc_split:
                        nc.sync.dma_start(
                            out=w_sb[:, :, 0:DC // 2], in_=wr[:, :, 0:DC // 2])
                        rests.append(lambda w_sb=w_sb, wr=wr: nc.sync.dma_start(
                            out=w_sb[:, :, DC // 2:], in_=wr[:, :, DC // 2:]))
                    else:
                        nc.sync.dma_start(out=w_sb, in_=wr)
                if fc_split:
                    rest = lambda: [r() for r in rests]
                    if defer:
                        return tuple(pair), rest
                    rest()
                return tuple(pair)

            def load_x(xT_dram, g, name, tag="x", bufs=None, split=False):
                pair = []
                srcs = []
                for i, xd in enumerate(xT_dram):
                    x_sb = xpool.tile(
                        [P, NDCH, QG], f8, tag=f"{tag}{i}", name=f"{name}_{i}",
                        bufs=bufs)
                    pair.append(x_sb)
                    srcs.append(
                        xd.rearrange("(c p) t -> p c t", p=P)[:, :, ts(g, QG)])
                if split:
                    h_ = NDCH // 2
                    for dsl in (slice(0, h_), slice(h_, NDCH)):
                        for x_sb, xr in zip(pair, srcs):
                            nc.sync.dma_start(
                                out=x_sb[:, dsl, :], in_=xr[:, dsl, :])
                else:
                    for x_sb, xr in zip(pair, srcs):
                        nc.sync.dma_start(out=x_sb, in_=xr)
                return tuple(pair)

            def proj_mms(ps, w_pair, x_pair, fc, half):
                """3-term hi/lo fp8 DoubleRow chain: w8*x8 + w8*x8l + w8l*x8.
                Contraction pairs c of 256 rows; 3 DR matmuls each."""
                w8, w8l = w_pair
                x8, x8l = x_pair
                cs = range(0, NDCH // 4) if half == 0 else (
                    range(NDCH // 4, NDCH // 2) if half == 1
                    else range(NDCH // 2))
                ncp = NDCH // 2
                for c in cs:
                    d = slice(2 * c, 2 * c + 2)
                    for t, (wt, xt) in enumerate(
                        ((w8, x8), (w8, x8l), (w8l, x8))
                    ):
                        nc.tensor.matmul(
                            ps, wt[:, d, ts(fc, P)], xt[:, d, :],
                            start=(c == 0 and t == 0),
                            stop=(c == ncp - 1 and t == 2),
                            perf_mode=DR,
                        )

            def kproj_chain(w_sb, x_sb, g, fc, half=None, state={}):
                if half in (None, 0):
                    state["ps"] = pp.tile(
                        [P, QG], f32, tag="pp", name=f"pk_{g}_{fc}"
                    )
                ps = state["ps"]
                proj_mms(ps, w_sb, x_sb, fc, half)
                if half in (None, 1):
                    nc.vector.tensor_scalar(
                        out=kT[:, fc, ts(g, QG)], in0=ps,
                        scalar1=bk_sb[:, fc : fc + 1], scalar2=None, op0=ALU.add,
                    )

            def qproj_chain(w_sb, x_sb, qT, g, fc, half=None, state={}):
                if half in (None, 0):
                    state["ps"] = pp.tile(
                        [P, QG], f32, tag="pp", name=f"pq_{g}_{fc}"
                    )
                ps = state["ps"]
                proj_mms(ps, w_sb, x_sb, fc, half)
                if half in (None, 1):
                    nc.vector.tensor_scalar(
                        out=qT[:, fc, :], in0=ps,
                        scalar1=bq_sb[:, fc : fc + 1], scalar2=None, op0=ALU.add,
                    )

            def vproj_tile(w_sb, x_sb, kt):
                tt = kt % NQT
                w8, w8l = w_sb
                x8, x8l = x_sb
                ps = pp.tile([P, DC], f32, tag="pp", name=f"pv_{kt}")
                ncp = NDCH // 2
                for c in range(ncp):
                    d = slice(2 * c, 2 * c + 2)
                    for t, (xt, wt) in enumerate(
                        ((x8, w8), (x8, w8l), (x8l, w8))
                    ):
                        nc.tensor.matmul(
                            ps, xt[:, d, ts(tt, P)], wt[:, d, :],
                            start=(c == 0 and t == 0),
                            stop=(c == ncp - 1 and t == 2),
                            perf_mode=DR,
                        )
                nc.vector.tensor_add(
                    out=v_aug[:, kt, :, 0:DK],
                    in0=ps.rearrange("p (h d) -> p h d", h=HC),
                    in1=bvb.rearrange("p (h d) -> p h d", h=HC),
                )

            def outproj_chain(attnT, g, tt, eg, pool=None, copy_act=False):
                pool = pool or pp
                ps = pool.tile(
                    [P, DC], f32, tag="pp" if pool is pp else "av",
                    name=f"po_{g}_{tt}_{eg}",
                )
                for fc in range(NFC):
                    nc.tensor.matmul(
                        ps, attnT[:, fc, ts(tt, P)], wo_sb[:, fc, ts(eg, DC)],
                        start=(fc == 0), stop=(fc == NFC - 1),
                    )
                o_sb = ocpool.tile([P, DC], bf16, tag="osb", name=f"ob_{g}_{tt}_{eg}")
                if copy_act:
                    nc.scalar.copy(out=o_sb, in_=ps)
                else:
                    nc.vector.tensor_copy(out=o_sb, in_=ps)
                nc.sync.dma_start(
                    out=out[ds(g * QG + tt * P, P), ts(eg, DC)], in_=o_sb
                )

            # ---- pair state: est tiles + av/z banks, consumed one pair later
            class PairState:
                def __init__(self, g, hp):
                    self.g, self.hp = g, hp
                    self.ests = {}   # h -> list of 8 est tiles [P, 2, QG]
                    self.av = None   # [P, NQT, 2, DK] f32 psum (1 bank)
                    self.zt = None   # [P, QG] f32 psum (1 bank; cols 0:8 used)

            def attv_slice(ps_, s):
                """att@V + Z matmuls consuming est[s] (key tiles 2s, 2s+1)."""
                g, hp = ps_.g, ps_.hp
                if s == 0:
                    ps_.av = avp.tile(
                        [P, NQT, 2, DK], f32, tag="av", name=f"av_{g}_{hp}"
                    )
                    ps_.zt = zpool.tile([P, QG], f32, tag="z", name=f"z_{g}_{hp}")
                last = NKT // 2 - 1
                for kk in range(2):
                    kt = 2 * s + kk
                    for qt in range(NQT):
                        for hh in range(2):
                            h = 2 * hp + hh
                            est = ps_.ests[h][s]
                            stat = est[:, kk, ts(qt, P)]
                            first = s == 0 and kk == 0 and qt == 0 and hh == 0
                            lastm = s == last and kk == 1 and qt == NQT - 1 and hh == 1
                            nc.tensor.matmul(
                                ps_.av[:, qt, hh, :], stat,
                                v_aug[:, kt, h, 0:DK],
                                start=first, stop=lastm,
                            )
                            c = qt * 2 + hh
                            nc.tensor.matmul(
                                ps_.zt[:, c : c + 1], stat,
                                v_aug[:, kt, h, DK:VW],
                                start=first, stop=lastm,
                            )

            def finish_pair(ps_, o_sb_tiles, qts=None):
                """reciprocal + normalize for a finished pair.

                qts: restrict the normalize to these q-tiles (tail
                pipelining); reciprocal runs only when qts is None or
                starts at qt 0."""
                g, hp = ps_.g, ps_.hp
                if qts is None or qts[0] == 0:
                    ps_.rz = spool.tile(
                        [P, NQT, 2], f32r, tag="rz", name=f"rz_{g}_{hp}"
                    )
                    with nc.allow_low_precision("softmax denom reciprocal"):
                        nc.vector.reciprocal(
                            out=ps_.rz,
                            in_=ps_.zt[:, 0 : 2 * NQT].rearrange(
                                "p (q h) -> p q h", q=NQT
                            ),
                        )
                o_sb = o_sb_tiles[g]
                if qts is None:
                    nc.vector.tensor_tensor(
                        out=o_sb[:, :, 2 * hp : 2 * hp + 2, :],
                        in0=ps_.av,
                        in1=ps_.rz.unsqueeze(-1).broadcast_to([P, NQT, 2, DK]),
                        op=ALU.mult,
                    )
                else:
                    for qt in qts:
                        nc.vector.tensor_tensor(
                            out=o_sb[:, qt, 2 * hp : 2 * hp + 2, :],
                            in0=ps_.av[:, qt, :, :],
                            in1=ps_.rz[:, qt, :].unsqueeze(-1).broadcast_to(
                                [P, 2, DK]),
                            op=ALU.mult,
                        )

            def transposes(g, o_sb_tiles, attnT):
                o_sb = o_sb_tiles[g]
                for qt in range(NQT):
                    for fc in range(NFC):
                        nc.sync.dma_start_transpose(
                            out=attnT[:, fc, ts(qt, P)],
                            in_=o_sb[:, qt, 2 * fc : 2 * fc + 2, :],
                        )

            # =========== prelude ===========
            # DMA order tuned so the first-score chain (wk fc01, xk0, wq
            # fc01, xq0) clears in ~10us and fill-phase consumers (xv0, wv,
            # xk1-3) arrive before their spliced chains need them.
            wk_sb, wk_rest = load_w(wk, "w_k", fc_split=True, defer=True)
            xk_sbs = [load_x(xkT, 0, "x_k_0", tag="xk", bufs=4, split=True)]
            wq_sb, wq_rest = load_w(wq, "w_q", tag="wq", bufs=1, fc_split=True,
                                    defer=True)
            xq_tiles = {0: load_x(xqT, 0, "x_q_0", tag="xq", bufs=2, split=True)}
            small_loads()
            kproj_chain(wk_sb, xk_sbs[0], 0, 0, half=0)
            kproj_chain(wk_sb, xk_sbs[0], 0, 0, half=1)

            qst = {0: qpool.tile([P, NFC, QG], bf16, tag="qT", name="qT_0")}
            qproj_chain(wq_sb, xq_tiles[0], qst[0], 0, 0, half=0)
            qproj_chain(wq_sb, xq_tiles[0], qst[0], 0, 0, half=1)

            xk_sbs.append(load_x(xkT, 1, "x_k_1", tag="xk", bufs=4))
            wv_sb = load_w(wv, "w_v")
            xv_tiles = {0: load_x(xvT, 0, "x_v_0", tag="xv", bufs=2)}
            xk_sbs.append(load_x(xkT, 2, "x_k_2", tag="xk", bufs=4))
            xk_sbs.append(load_x(xkT, 3, "x_k_3", tag="xk", bufs=4))
            wk_rest()
            wq_rest()
            nc.sync.dma_start(out=wo_sb, in_=wo.rearrange("(c p) e -> p c e", p=P))

            # =========== splice schedule ===========
            # pair index p = 4*g + hp runs score loop slots 0..7; sched[p][s]
            # is a list of thunks emitted before slot s's score matmuls.
            sched = {p: {s: [] for s in range(8)} for p in range(16)}

            def at(p, s, fn):
                sched[p][s].append(fn)

            # kproj: fc=0 for kg>=1 early in pair 0; fc=f in pair f-1... but
            # pair (0,hp) reads kT chunk hp for all kt: chunk fc must be fully
            # projected (all 4 kg) before pair (0,fc) starts.
            for kg, s_ in [(1, 0), (2, 1), (3, 3)]:
                at(0, s_, lambda kg=kg: kproj_chain(wk_sb, xk_sbs[kg], kg, 0))
            for fc in range(1, 4):
                for kg, (p_, s_) in enumerate(
                    [(fc - 1, 3), (fc - 1, 5), (fc, 0), (fc, 2)]
                ):
                    at(p_, s_, lambda kg=kg, fc=fc: kproj_chain(
                        wk_sb, xk_sbs[kg], kg, fc))
            # vproj: 10 tiles in pair 0 (extra on later slots), 6 in pair 1;
            # v_aug[kt] needed by attV(0,0) slice s=kt//2 at pair 1 slot s.
            # xv loads run >=2 slots ahead of their first vproj consumer.
            for vg, (p_, s_) in {1: (0, 1), 2: (0, 5), 3: (0, 7)}.items():
                at(p_, s_, lambda vg=vg: xv_tiles.__setitem__(
                    vg, load_x(xvT, vg, f"x_v_{vg}", tag="xv", bufs=2)))
            vq = [(0, 0, 1), (0, 1, 1), (0, 2, 1), (0, 3, 1), (0, 4, 1),
                  (0, 5, 1), (0, 6, 1), (0, 7, 1), (1, 0, 2), (1, 1, 2),
                  (1, 2, 2), (1, 3, 2)]
            kt_next = 0
            for p_, s_, n_ in vq:
                for _ in range(n_):
                    if kt_next >= NKT:
                        break
                    kt = kt_next
                    kt_next += 1
                    at(p_, s_, lambda kt=kt: vproj_tile(
                        wv_sb, xv_tiles[kt // NQT], kt))
            # qproj for pair p+1 at pair p slot 5 (+ xq loads 2 pairs early)
            for p in range(15):
                g1, fc1 = divmod(p + 1, 4)
                if fc1 == 0 and g1 > 0:
                    at(p - 2 if p >= 2 else 0, 1, lambda g1=g1: xq_tiles.__setitem__(
                        g1, load_x(xqT, g1, f"x_q_{g1}", tag="xq", bufs=2)))
                    at(p, 5, lambda g1=g1: (
                        qst.__setitem__(g1, qpool.tile(
                            [P, NFC, QG], bf16, tag="qT", name=f"qT_{g1}")),
                        qproj_chain(wq_sb, xq_tiles[g1], qst[g1], g1, 0,
                                    half=0))[-1])
                    at(p, 7, lambda g1=g1: qproj_chain(
                        wq_sb, xq_tiles[g1], qst[g1], g1, 0, half=1))
                else:
                    at(p, 5, lambda g1=g1, fc1=fc1: qproj_chain(
                        wq_sb, xq_tiles[g1], qst[g1], g1, fc1, half=0))
                    at(p, 7, lambda g1=g1, fc1=fc1: qproj_chain(
                        wq_sb, xq_tiles[g1], qst[g1], g1, fc1, half=1))
            # outproj(g) chains spliced into pairs of group g+1
            op_slots = [(1, 4), (1, 6), (2, 2), (2, 4), (2, 6), (3, 2),
                        (3, 4), (3, 6)]
            attnT_holder = {}
            for g in range(3):
                for i, (hp_, s_) in enumerate(op_slots):
                    tt, eg = divmod(i, 2)
                    at(4 * (g + 1) + hp_, s_, lambda g=g, tt=tt, eg=eg: outproj_chain(
                        attnT_holder[g], g, tt, eg))

            # =========== main loop ===========
            o_sb_tiles = {}
            prev_pair = None   # PairState consumed by current pair's splices
            done_pair = None   # PairState whose attV completed last pair
            # (its finish_pair runs at the START of this pair so the DVE
            # queue never parks on unmet deps — DVE is in-order)

            for p in range(16):
                g, hp = divmod(p, 4)
                if g not in o_sb_tiles:
                    o_sb_tiles[g] = ospool.tile(
                        [P, NQT, HC, DK], bf16, tag="osb2", name=f"o_{g}"
                    )
                cur = PairState(g, hp)
                qT = qst[g]
                for kt2 in range(NKT // 2):
                    if kt2 == 0 and done_pair is not None:
                        finish_pair(done_pair, o_sb_tiles)
                        if done_pair.hp == NHP - 1:
                            gg = done_pair.g
                            attnT_holder[gg] = atpool.tile(
                                [P, NFC, QG], bf16, tag="attnT", name=f"aT_{gg}"
                            )
                            transposes(gg, o_sb_tiles, attnT_holder[gg])
                        done_pair = None
                    def emit_splices():
                        if prev_pair is not None:
                            attv_slice(prev_pair, kt2)
                        for fn in sched[p][kt2]:
                            fn()

                    def emit_scores():
                        sts = {}
                        for hh in range(2):
                            h = 2 * hp + hh
                            sts[h] = st_pool.tile(
                                [P, 2, QG], f32, tag="st",
                                name=f"st_{g}_{h}_{kt2}"
                            )
                        for kk in range(2):
                            kt = 2 * kt2 + kk
                            for hh in range(2):
                                h = 2 * hp + hh
                                r0 = hh * DK
                                nc.tensor.matmul(
                                    sts[h][:, kk, :],
                                    kT[r0 : r0 + DK, hp, ts(kt, P)],
                                    qT[r0 : r0 + DK, hp, :],
                                    start=True, stop=True,
                                    tile_position=(r0, 0),
                                )
                        for hh in range(2):
                            h = 2 * hp + hh
                            e = epool.tile(
                                [P, 2, QG], bf16, tag="est",
                                name=f"est_{g}_{h}_{kt2}"
                            )
                            cur.ests.setdefault(h, []).append(e)
                            if hh == 1 and kt2 % 2 == 1:
                                # Schraudolph exp on DVE: bf16 bit pattern of
                                # exp(s*INV_SCALE) ~= trunc(A*s + B); offloads
                                # 25% of the exp stream from ScalarE (max rel
                                # err ~3%, partially cancelled by softmax)
                                nc.vector.tensor_scalar(
                                    out=e.bitcast(mybir.dt.int16),
                                    in0=sts[h],
                                    scalar1=SCH_A, scalar2=SCH_B,
                                    op0=ALU.mult, op1=ALU.add,
                                )
                            else:
                                nc.scalar.activation(
                                    out=e, in_=sts[h], func=AF.Exp,
                                    scale=INV_SCALE
                                )

                    # fill phase (pairs 0-2): ACT is starved, so feed it
                    # scores before the heavy projection splices; steady
                    # state: splices first (PE uses the st-ring wait time)
                    if p < 3:
                        emit_scores()
                        emit_splices()
                    else:
                        emit_splices()
                        emit_scores()
                # previous pair's attV is complete; finish it at the start
                # of the next pair (deps met there, no DVE queue parking)
                done_pair = prev_pair
                prev_pair = cur

            # =========== tail: last pair's attV + outproj of group 3 ====
            # per-qt pipelining: as soon as qt's normalize lands, its
            # transposes, outproj chains and output DMA flow while the PE
            # works the next qt.
            finish_pair(done_pair, o_sb_tiles)
            for s in range(NKT // 2):
                attv_slice(prev_pair, s)
            attnT_holder[3] = atpool.tile(
                [P, NFC, QG], bf16, tag="attnT", name="aT_3"
            )
            o_sb3 = o_sb_tiles[3]
            for qt in range(NQT):
                finish_pair(prev_pair, o_sb_tiles, qts=[qt])
                for fc in range(NFC):
                    nc.sync.dma_start_transpose(
                        out=attnT_holder[3][:, fc, ts(qt, P)],
                        in_=o_sb3[:, qt, 2 * fc : 2 * fc + 2, :],
                    )
                for eg in range(2):
                    outproj_chain(attnT_holder[3], 3, qt, eg, copy_act=True)

    nc.compile()
    return nc


def _get_nc(debug=False):
    if "nc" not in _CACHE:
        _CACHE["nc"] = _build()
    return _CACHE["nc"]


def _tf32(a):
    """Round fp32 to the TF32 grid (10-bit mantissa, round-to-nearest-even)."""
    u = np.ascontiguousarray(a, dtype=np.float32).view(np.uint32)
    u = (u + np.uint32(0xFFF) + ((u >> np.uint32(13)) & np.uint32(1))) & np.uint32(
        0xFFFFE000
    )
    return u.view(np.float32)


def _bf16(a):
    import ml_dtypes

    return np.ascontiguousarray(a, dtype=np.float32).astype(ml_dtypes.bfloat16)


def _make_in_maps(inputs):
    q = np.asarray(inputs["query"], dtype=np.float32)
    k = np.asarray(inputs["key"], dtype=np.float32)
    v = np.asarray(inputs["value"], dtype=np.float32)
    wq = np.asarray(inputs["wq"], dtype=np.float32)
    wk = np.asarray(inputs["wk"], dtype=np.float32)
    wv = np.asarray(inputs["wv"], dtype=np.float32)
    wo = np.asarray(inputs["wo"], dtype=np.float32)
    bq = np.asarray(inputs["bq"], dtype=np.float32)
    bk = np.asarray(inputs["bk"], dtype=np.float32)
    bv = np.asarray(inputs["bv"], dtype=np.float32)

    import ml_dtypes

    def _hl(a):
        hi = np.ascontiguousarray(a, dtype=np.float32).astype(
            ml_dtypes.float8_e4m3)
        lo = (a - hi.astype(np.float32)).astype(ml_dtypes.float8_e4m3)
        return hi, lo

    WS = 32.0  # fp8 weight pre-scale (undone via exp scale / ones column)
    xT = [(_hl(q[b].T), _hl(k[b].T), _hl(v[b].T)) for b in range(B)]
    in_maps = []
    for c in range(NCORES):
        b, g = divmod(c, 2)
        sl = slice(g * DC, (g + 1) * DC)
        wq8, wq8l = _hl(wq[:, sl] * WS)
        wk8, wk8l = _hl(wk[:, sl] * WS)
        wv8, wv8l = _hl(wv[:, sl] * WS)
        in_maps.append(
            {
                "xq8": xT[b][0][0], "xq8l": xT[b][0][1],
                "xk8": xT[b][1][0], "xk8l": xT[b][1][1],
                "xv8": xT[b][2][0], "xv8l": xT[b][2][1],
                "wq8": wq8, "wq8l": wq8l,
                "wk8": wk8, "wk8l": wk8l,
                "wv8": wv8, "wv8l": wv8l,
                "wo": _bf16(wo[sl, :]),
                "bq": np.ascontiguousarray(bq[sl] * WS),
                "bk": np.ascontiguousarray(bk[sl] * WS),
                "bv": np.ascontiguousarray(bv[sl] * WS),
            }
        )
    return in_maps


def run(inputs, **kwargs):
    """Run the kernel; returns (full_output, BassKernelResults)."""
    from concourse.bass_utils import run_bass_kernel_spmd

    kwargs.pop("debug", None)
    nc = _get_nc()
    in_maps = _make_in_maps(inputs)
    res = run_bass_kernel_spmd(nc, in_maps, core_ids=list(range(NCORES)), **kwargs)
    bo = np.asarray(inputs["bo"], dtype=np.float32)
    final = np.empty((B, S, D), np.float32)
    for b in range(B):
        final[b] = (
            res.results[2 * b]["out"].astype(np.float32)
            + res.results[2 * b + 1]["out"].astype(np.float32)
            + bo
        )
    return final, res


def kernel(**inputs):
    return run(inputs)[0]


# revision 27
# speedup vs baseline: 1.0262x; 1.0154x over previous
"""Multi-head attention (B=4, S=2048, D=1024, H=16) on 8 TRN2 NeuronCores.

Sharding (Megatron-style, per spec hint): data-parallel over batch (4) x
tensor-parallel over heads (2 groups of 8). Core c handles batch c//2,
head-group c%2. QKV projections column-sharded, output projection
row-sharded; the two partial bf16 outputs per batch are summed on the host
together with the output bias.

Per-core kernel (one NeuronCore, 8 heads, 2048 tokens):
  - QKV projections run as fp8e4 DoubleRow matmuls (256-deep contraction at
    0.5 cyc/row): weights are pre-scaled x32 (so the lo residual stays out
    of e4m3's subnormal range) and split hi/lo host-side, x likewise; the
    three products w8*x8 + w8*x8l + w8l*x8 give ~9-bit effective precision
    at 0.75x the bf16 instruction cost. The x32^2 score scale is undone in
    the exp; the v-side x32 cancels via the Z column (ones column = 32).
  - Scores transposed ST[k, q]; softmax-exp without max-subtraction, one
    ACT pass per [128, 2, 512] tile -> bf16. A quarter of the exp tiles
    (kt2 odd, second head of each pair) run on the DVE instead as a
    Schraudolph bit-trick (bf16 bits = trunc(A*s + B), max rel err ~3%,
    softmax-cancelled), offloading the ScalarE bottleneck.
  - att@V uses the probabilities as the STATIONARY operand ([128k, 128q]
    slices) and v tiles [128k, 64] as moving, so the output [128q, 64]
    fills all 128 PSUM partitions (half the PE cost of the v-stationary
    form). A head-pair's whole output (4 qt x 2 h x 64) packs into exactly
    one PSUM bank with a single accumulation start/stop; Z accumulates via
    1-column matmuls against the v_aug ones column into a z bank.
  - Normalization is one DVE pass per pair (stride-0 broadcast of 1/Z);
    the normalized [q, feature] tiles go back to feature-major via the DMA
    xbar (dma_start_transpose), not the PE.
  - att@V chains are spliced into the NEXT pair's score loop; k/v/q
    projections and the previous group's output projection are spliced the
    same way (deadline-scheduled), so ScalarE/DVE stream exps with few
    gaps. finish_pair (recip+normalize) is emitted one pair late so the
    in-order DVE queue never parks on unmet deps. The tail pipelines
    per-q-tile: normalize -> xbar transpose -> outproj -> bf16 store.
"""

import sys

if "/opt/trn_rl_repo" not in sys.path:
    sys.path.insert(0, "/opt/trn_rl_repo")

import numpy as np

B, S, D = 4, 2048, 1024
H, DK = 16, 64
NCORES = 8
HC = H // 2            # heads per core
DC = HC * DK           # 512 local features per core
INV_SCALE = 1.0 / 8.0 / (32.0 * 32.0)  # 1/sqrt(DK), /32^2 fp8 weight scale
P = 128
NDCH = D // P          # 8 contraction chunks for projections
NFC = DC // P          # 4 local feature chunks
NKT = S // P           # 16 key tiles
NQG = 4                # query groups
QG = S // NQG          # 512 queries per group
NQT = QG // P          # 4 query tiles per group
VW = DK + 1            # 65: v columns + ones column
NHP = HC // 2          # head pairs

_CACHE = {}


def _build():
    import concourse.bass as bass
    import concourse.bacc as bacc
    import concourse.tile as tile
    import concourse.mybir as mybir
    from concourse.bass import ts, ds

    f32 = mybir.dt.float32
    f32r = mybir.dt.float32r
    bf16 = mybir.dt.bfloat16
    AF = mybir.ActivationFunctionType
    ALU = mybir.AluOpType

    LOG2E = 1.4426950408889634
    SCH_A = INV_SCALE * LOG2E * 128.0
    SCH_B = 16256.0 - 5.5 + 0.5  # centering + trunc->round bias

    nc = bacc.Bacc("TRN2", target_bir_lowering=False, num_devices=NCORES)

    f8 = mybir.dt.float8e4
    DR = mybir.MatmulPerfMode.DoubleRow
    xqT = (nc.dram_tensor("xq8", [D, S], f8, kind="ExternalInput"),
           nc.dram_tensor("xq8l", [D, S], f8, kind="ExternalInput"))
    xkT = (nc.dram_tensor("xk8", [D, S], f8, kind="ExternalInput"),
           nc.dram_tensor("xk8l", [D, S], f8, kind="ExternalInput"))
    xvT = (nc.dram_tensor("xv8", [D, S], f8, kind="ExternalInput"),
           nc.dram_tensor("xv8l", [D, S], f8, kind="ExternalInput"))
    wq = (nc.dram_tensor("wq8", [D, DC], f8, kind="ExternalInput"),
          nc.dram_tensor("wq8l", [D, DC], f8, kind="ExternalInput"))
    wk = (nc.dram_tensor("wk8", [D, DC], f8, kind="ExternalInput"),
          nc.dram_tensor("wk8l", [D, DC], f8, kind="ExternalInput"))
    wv = (nc.dram_tensor("wv8", [D, DC], f8, kind="ExternalInput"),
          nc.dram_tensor("wv8l", [D, DC], f8, kind="ExternalInput"))
    wo = nc.dram_tensor("wo", [DC, D], bf16, kind="ExternalInput")
    bq = nc.dram_tensor("bq", [DC], f32, kind="ExternalInput")
    bk = nc.dram_tensor("bk", [DC], f32, kind="ExternalInput")
    bv = nc.dram_tensor("bv", [DC], f32, kind="ExternalInput")
    out = nc.dram_tensor("out", [S, D], bf16, kind="ExternalOutput")

    with tile.TileContext(nc) as tc:
        with (
            tc.tile_pool(name="persist", bufs=1) as persist,
            tc.tile_pool(name="wts", bufs=2) as wpool,
            tc.tile_pool(name="xin", bufs=4) as xpool,
            tc.tile_pool(name="qt", bufs=2) as qpool,
            tc.tile_pool(name="expst", bufs=22) as epool,
            tc.tile_pool(name="osb", bufs=2) as ospool,
            tc.tile_pool(name="att", bufs=1) as atpool,
            tc.tile_pool(name="small", bufs=2) as spool,
            tc.tile_pool(name="oc", bufs=4) as ocpool,
            tc.tile_pool(name="pp", bufs=2, space="PSUM") as pp,
            tc.tile_pool(name="st", bufs=2, space="PSUM") as st_pool,
            tc.tile_pool(name="av", bufs=1, space="PSUM") as avp,
            tc.tile_pool(name="zp", bufs=1, space="PSUM") as zpool,
        ):
            # ---- persistent SBUF tensors ----
            kT = persist.tile([P, NFC, S], bf16)          # 16KB/part
            v_aug = persist.tile([P, NKT, HC, VW], bf16)  # ~16.6KB/part
            wo_sb = persist.tile([P, NFC, D], bf16)       # 8KB/part
            bq_sb = persist.tile([P, NFC], f32)
            bk_sb = persist.tile([P, NFC], f32)
            bvb = persist.tile([P, DC], f32)              # bias_v broadcast

            def small_loads():
                nc.sync.dma_start(
                    out=bq_sb, in_=bq.rearrange("(c p) -> p c", p=P))
                nc.sync.dma_start(
                    out=bk_sb, in_=bk.rearrange("(c p) -> p c", p=P))
                bv_ap = bv.ap()
                bvb_src = bass.AP(
                    tensor=bv_ap.tensor, offset=bv_ap.offset,
                    ap=[[0, P], *bv_ap.ap]
                )
                nc.sync.dma_start(out=bvb, in_=bvb_src)
            # ones column of v_aug (softmax denominator trick)
            ones_st = persist.tile([P, P], f32)
            nc.vector.memset(ones_st, 32.0)
            nc.vector.tensor_copy(
                out=v_aug[:, :, :, DK],
                in_=ones_st.rearrange("p (k h) -> p k h", k=NKT),
            )

            # ---- emission helpers (PE program order == emission order) ----
            def load_w(w_dram, name, tag="w", bufs=None, fc_split=False,
                       defer=False):
                pair = []
                for i, wd in enumerate(w_dram):
                    pair.append(wpool.tile(
                        [P, NDCH, DC], f8, tag=f"{tag}{i}", name=f"{name}_{i}",
                        bufs=bufs))
                rests = []
                for w_sb, wd in zip(pair, w_dram):
                    wr = wd.rearrange("(c p) f -> p c f", p=P)
                    if fc_split:
                        nc.sync.dma_start(
                            out=w_sb[:, :, 0:DC // 2], in_=wr[:, :, 0:DC // 2])
                        rests.append(lambda w_sb=w_sb, wr=wr: nc.sync.dma_start(
                            out=w_sb[:, :, DC // 2:], in_=wr[:, :, DC // 2:]))
                    else:
                        nc.sync.dma_start(out=w_sb, in_=wr)
                if fc_split:
                    rest = lambda: [r() for r in rests]
                    if defer:
                        return tuple(pair), rest
                    rest()
                return tuple(pair)

            def load_x(xT_dram, g, name, tag="x", bufs=None, split=False):
                pair = []
                srcs = []
                for i, xd in enumerate(xT_dram):
                    x_sb = xpool.tile(
                        [P, NDCH, QG], f8, tag=f"{tag}{i}", name=f"{name}_{i}",
                        bufs=bufs)
                    pair.append(x_sb)
                    srcs.append(
                        xd.rearrange("(c p) t -> p c t", p=P)[:, :, ts(g, QG)])
                if split:
                    h_ = NDCH // 2
                    for dsl in (slice(0, h_), slice(h_, NDCH)):
                        for x_sb, xr in zip(pair, srcs):
                            nc.sync.dma_start(
                                out=x_sb[:, dsl, :], in_=xr[:, dsl, :])
                else:
                    for x_sb, xr in zip(pair, srcs):
                        nc.sync.dma_start(out=x_sb, in_=xr)
                return tuple(pair)

            def proj_mms(ps, w_pair, x_pair, fc, half):
                """3-term hi/lo fp8 DoubleRow chain: w8*x8 + w8*x8l + w8l*x8.
                Contraction pairs c of 256 rows; 3 DR matmuls each."""
                w8, w8l = w_pair
                x8, x8l = x_pair
                cs = range(0, NDCH // 4) if half == 0 else (
                    range(NDCH // 4, NDCH // 2) if half == 1
                    else range(NDCH // 2))
                ncp = NDCH // 2
                for c in cs:
                    d = slice(2 * c, 2 * c + 2)
                    for t, (wt, xt) in enumerate(
                        ((w8, x8), (w8, x8l), (w8l, x8))
                    ):
                        nc.tensor.matmul(
                            ps, wt[:, d, ts(fc, P)], xt[:, d, :],
                            start=(c == 0 and t == 0),
                            stop=(c == ncp - 1 and t == 2),
                            perf_mode=DR,
                        )

            def kproj_chain(w_sb, x_sb, g, fc, half=None, state={}):
                if half in (None, 0):
                    state["ps"] = pp.tile(
                        [P, QG], f32, tag="pp", name=f"pk_{g}_{fc}"
                    )
                ps = state["ps"]
                proj_mms(ps, w_sb, x_sb, fc, half)
                if half in (None, 1):
                    nc.vector.tensor_scalar(
                        out=kT[:, fc, ts(g, QG)], in0=ps,
                        scalar1=bk_sb[:, fc : fc + 1], scalar2=None, op0=ALU.add,
                    )

            def qproj_chain(w_sb, x_sb, qT, g, fc, half=None, state={}):
                if half in (None, 0):
                    state["ps"] = pp.tile(
                        [P, QG], f32, tag="pp", name=f"pq_{g}_{fc}"
                    )
                ps = state["ps"]
                proj_mms(ps, w_sb, x_sb, fc, half)
                if half in (None, 1):
                    nc.vector.tensor_scalar(
                        out=qT[:, fc, :], in0=ps,
                        scalar1=bq_sb[:, fc : fc + 1], scalar2=None, op0=ALU.add,
                    )

            def vproj_tile(w_sb, x_sb, kt):
                tt = kt % NQT
                w8, w8l = w_sb
                x8, x8l = x_sb
                ps = pp.tile([P, DC], f32, tag="pp", name=f"pv_{kt}")
                ncp = NDCH // 2
                for c in range(ncp):
                    d = slice(2 * c, 2 * c + 2)
                    for t, (xt, wt) in enumerate(
                        ((x8, w8), (x8, w8l), (x8l, w8))
                    ):
                        nc.tensor.matmul(
                            ps, xt[:, d, ts(tt, P)], wt[:, d, :],
                            start=(c == 0 and t == 0),
                            stop=(c == ncp - 1 and t == 2),
                            perf_mode=DR,
                        )
                nc.vector.tensor_add(
                    out=v_aug[:, kt, :, 0:DK],
                    in0=ps.rearrange("p (h d) -> p h d", h=HC),
                    in1=bvb.rearrange("p (h d) -> p h d", h=HC),
                )

            def outproj_chain(attnT, g, tt, eg, pool=None, copy_act=False):
                pool = pool or pp
                ps = pool.tile(
                    [P, DC], f32, tag="pp" if pool is pp else "av",
                    name=f"po_{g}_{tt}_{eg}",
                )
                for fc in range(NFC):
                    nc.tensor.matmul(
                        ps, attnT[:, fc, ts(tt, P)], wo_sb[:, fc, ts(eg, DC)],
                        start=(fc == 0), stop=(fc == NFC - 1),
                    )
                o_sb = ocpool.tile([P, DC], bf16, tag="osb", name=f"ob_{g}_{tt}_{eg}")
                if copy_act:
                    nc.scalar.copy(out=o_sb, in_=ps)
                else:
                    nc.vector.tensor_copy(out=o_sb, in_=ps)
                nc.sync.dma_start(
                    out=out[ds(g * QG + tt * P, P), ts(eg, DC)], in_=o_sb
                )

            # ---- pair state: est tiles + av/z banks, consumed one pair later
            class PairState:
                def __init__(self, g, hp):
                    self.g, self.hp = g, hp
                    self.ests = {}   # h -> list of 8 est tiles [P, 2, QG]
                    self.av = None   # [P, NQT, 2, DK] f32 psum (1 bank)
                    self.zt = None   # [P, QG] f32 psum (1 bank; cols 0:8 used)

            def attv_slice(ps_, s):
                """att@V + Z matmuls consuming est[s] (key tiles 2s, 2s+1)."""
                g, hp = ps_.g, ps_.hp
                if s == 0:
                    ps_.av = avp.tile(
                        [P, NQT, 2, DK], f32, tag="av", name=f"av_{g}_{hp}"
                    )
                    ps_.zt = zpool.tile([P, QG], f32, tag="z", name=f"z_{g}_{hp}")
                last = NKT // 2 - 1
                for kk in range(2):
                    kt = 2 * s + kk
                    for qt in range(NQT):
                        for hh in range(2):
                            h = 2 * hp + hh
                            est = ps_.ests[h][s]
                            stat = est[:, kk, ts(qt, P)]
                            first = s == 0 and kk == 0 and qt == 0 and hh == 0
                            lastm = s == last and kk == 1 and qt == NQT - 1 and hh == 1
                            nc.tensor.matmul(
                                ps_.av[:, qt, hh, :], stat,
                                v_aug[:, kt, h, 0:DK],
                                start=first, stop=lastm,
                            )
                            c = qt * 2 + hh
                            nc.tensor.matmul(
                                ps_.zt[:, c : c + 1], stat,
                                v_aug[:, kt, h, DK:VW],
                                start=first, stop=lastm,
                            )

            def finish_pair(ps_, o_sb_tiles, qts=None):
                """reciprocal + normalize for a finished pair.

                qts: restrict the normalize to these q-tiles (tail
                pipelining); reciprocal runs only when qts is None or
                starts at qt 0."""
                g, hp = ps_.g, ps_.hp
                if qts is None or qts[0] == 0:
                    ps_.rz = spool.tile(
                        [P, NQT, 2], f32r, tag="rz", name=f"rz_{g}_{hp}"
                    )
                    with nc.allow_low_precision("softmax denom reciprocal"):
                        nc.vector.reciprocal(
                            out=ps_.rz,
                            in_=ps_.zt[:, 0 : 2 * NQT].rearrange(
                                "p (q h) -> p q h", q=NQT
                            ),
                        )
                o_sb = o_sb_tiles[g]
                if qts is None:
                    nc.vector.tensor_tensor(
                        out=o_sb[:, :, 2 * hp : 2 * hp + 2, :],
                        in0=ps_.av,
                        in1=ps_.rz.unsqueeze(-1).broadcast_to([P, NQT, 2, DK]),
                        op=ALU.mult,
                    )
                else:
                    for qt in qts:
                        nc.vector.tensor_tensor(
                            out=o_sb[:, qt, 2 * hp : 2 * hp + 2, :],
                            in0=ps_.av[:, qt, :, :],
                            in1=ps_.rz[:, qt, :].unsqueeze(-1).broadcast_to(
                                [P, 2, DK]),
                            op=ALU.mult,
                        )

            def transposes(g, o_sb_tiles, attnT):
                o_sb = o_sb_tiles[g]
                for qt in range(NQT):
                    for fc in range(NFC):
                        nc.sync.dma_start_transpose(
                            out=attnT[:, fc, ts(qt, P)],
                            in_=o_sb[:, qt, 2 * fc : 2 * fc + 2, :],
                        )

            # =========== prelude ===========
            # DMA order tuned so the first-score chain (wk fc01, xk0, wq
            # fc01, xq0) clears in ~10us and fill-phase consumers (xv0, wv,
            # xk1-3) arrive before their spliced chains need them.
            wk_sb, wk_rest = load_w(wk, "w_k", fc_split=True, defer=True)
            xk_sbs = [load_x(xkT, 0, "x_k_0", tag="xk", bufs=4, split=True)]
            wq_sb, wq_rest = load_w(wq, "w_q", tag="wq", bufs=1, fc_split=True,
                                    defer=True)
            xq_tiles = {0: load_x(xqT, 0, "x_q_0", tag="xq", bufs=2, split=True)}
            small_loads()
            kproj_chain(wk_sb, xk_sbs[0], 0, 0, half=0)
            kproj_chain(wk_sb, xk_sbs[0], 0, 0, half=1)

            qst = {0: qpool.tile([P, NFC, QG], bf16, tag="qT", name="qT_0")}
            qproj_chain(wq_sb, xq_tiles[0], qst[0], 0, 0, half=0)
            qproj_chain(wq_sb, xq_tiles[0], qst[0], 0, 0, half=1)

            xk_sbs.append(load_x(xkT, 1, "x_k_1", tag="xk", bufs=4))
            wv_sb = load_w(wv, "w_v")
            xv_tiles = {0: load_x(xvT, 0, "x_v_0", tag="xv", bufs=2)}
            xk_sbs.append(load_x(xkT, 2, "x_k_2", tag="xk", bufs=4))
            xk_sbs.append(load_x(xkT, 3, "x_k_3", tag="xk", bufs=4))
            wk_rest()
            wq_rest()
            nc.sync.dma_start(out=wo_sb, in_=wo.rearrange("(c p) e -> p c e", p=P))

            # =========== splice schedule ===========
            # pair index p = 4*g + hp runs score loop slots 0..7; sched[p][s]
            # is a list of thunks emitted before slot s's score matmuls.
            sched = {p: {s: [] for s in range(8)} for p in range(16)}

            def at(p, s, fn):
                sched[p][s].append(fn)

            # kproj: fc=0 for kg>=1 early in pair 0; fc=f in pair f-1... but
            # pair (0,hp) reads kT chunk hp for all kt: chunk fc must be fully
            # projected (all 4 kg) before pair (0,fc) starts.
            for kg, s_ in [(1, 0), (2, 1), (3, 3)]:
                at(0, s_, lambda kg=kg: kproj_chain(wk_sb, xk_sbs[kg], kg, 0))
            for fc in range(1, 4):
                for kg, (p_, s_) in enumerate(
                    [(fc - 1, 3), (fc - 1, 5), (fc, 0), (fc, 2)]
                ):
                    at(p_, s_, lambda kg=kg, fc=fc: kproj_chain(
                        wk_sb, xk_sbs[kg], kg, fc))
            # vproj: 10 tiles in pair 0 (extra on later slots), 6 in pair 1;
            # v_aug[kt] needed by attV(0,0) slice s=kt//2 at pair 1 slot s.
            # xv loads run >=2 slots ahead of their first vproj consumer.
            for vg, (p_, s_) in {1: (0, 1), 2: (0, 5), 3: (0, 7)}.items():
                at(p_, s_, lambda vg=vg: xv_tiles.__setitem__(
                    vg, load_x(xvT, vg, f"x_v_{vg}", tag="xv", bufs=2)))
            vq = [(0, 0, 1), (0, 1, 1), (0, 2, 1), (0, 3, 1), (0, 4, 1),
                  (0, 5, 1), (0, 6, 1), (0, 7, 1), (1, 0, 2), (1, 1, 2),
                  (1, 2, 2), (1, 3, 2)]
            kt_next = 0
            for p_, s_, n_ in vq:
                for _ in range(n_):
                    if kt_next >= NKT:
                        break
                    kt = kt_next
                    kt_next += 1
                    at(p_, s_, lambda kt=kt: vproj_tile(
                        wv_sb, xv_tiles[kt // NQT], kt))
            # qproj for pair p+1 at pair p slot 5 (+ xq loads 2 pairs early)
            for p in range(15):
                g1, fc1 = divmod(p + 1, 4)
                if fc1 == 0 and g1 > 0:
                    at(p - 2 if p >= 2 else 0, 1, lambda g1=g1: xq_tiles.__setitem__(
                        g1, load_x(xqT, g1, f"x_q_{g1}", tag="xq", bufs=2)))
                    at(p, 5, lambda g1=g1: (
                        qst.__setitem__(g1, qpool.tile(
                            [P, NFC, QG], bf16, tag="qT", name=f"qT_{g1}")),
                        qproj_chain(wq_sb, xq_tiles[g1], qst[g1], g1, 0,
                                    half=0))[-1])
                    at(p, 7, lambda g1=g1: qproj_chain(
                        wq_sb, xq_tiles[g1], qst[g1], g1, 0, half=1))
                else:
                    at(p, 5, lambda g1=g1, fc1=fc1: qproj_chain(
                        wq_sb, xq_tiles[g1], qst[g1], g1, fc1, half=0))
                    at(p, 7, lambda g1=g1, fc1=fc1: qproj_chain(
                        wq_sb, xq_tiles[g1], qst[g1], g1, fc1, half=1))
            # outproj(g) chains spliced into pairs of group g+1
            op_slots = [(1, 4), (1, 6), (2, 2), (2, 4), (2, 6), (3, 2),
                        (3, 4), (3, 6)]
            attnT_holder = {}
            for g in range(3):
                for i, (hp_, s_) in enumerate(op_slots):
                    tt, eg = divmod(i, 2)
                    at(4 * (g + 1) + hp_, s_, lambda g=g, tt=tt, eg=eg: outproj_chain(
                        attnT_holder[g], g, tt, eg))

            # =========== main loop ===========
            o_sb_tiles = {}
            prev_pair = None   # PairState consumed by current pair's splices
            done_pair = None   # PairState whose attV completed last pair
            # (its finish_pair runs at the START of this pair so the DVE
            # queue never parks on unmet deps — DVE is in-order)

            for p in range(16):
                g, hp = divmod(p, 4)
                if g not in o_sb_tiles:
                    o_sb_tiles[g] = ospool.tile(
                        [P, NQT, HC, DK], bf16, tag="osb2", name=f"o_{g}"
                    )
                cur = PairState(g, hp)
                qT = qst[g]
                for kt2 in range(NKT // 2):
                    if kt2 == 0 and done_pair is not None:
                        finish_pair(done_pair, o_sb_tiles)
                        if done_pair.hp == NHP - 1:
                            gg = done_pair.g
                            attnT_holder[gg] = atpool.tile(
                                [P, NFC, QG], bf16, tag="attnT", name=f"aT_{gg}"
                            )
                            transposes(gg, o_sb_tiles, attnT_holder[gg])
                        done_pair = None
                    def emit_splices():
                        if prev_pair is not None:
                            attv_slice(prev_pair, kt2)
                        for fn in sched[p][kt2]:
                            fn()

                    def emit_scores():
                        sts = {}
                        for hh in range(2):
                            h = 2 * hp + hh
                            sts[h] = st_pool.tile(
                                [P, 2, QG], f32, tag="st",
                                name=f"st_{g}_{h}_{kt2}"
                            )
                        for kk in range(2):
                            kt = 2 * kt2 + kk
                            for hh in range(2):
                                h = 2 * hp + hh
                                r0 = hh * DK
                                nc.tensor.matmul(
                                    sts[h][:, kk, :],
                                    kT[r0 : r0 + DK, hp, ts(kt, P)],
                                    qT[r0 : r0 + DK, hp, :],
                                    start=True, stop=True,
                                    tile_position=(r0, 0),
                                )
                        for hh in range(2):
                            h = 2 * hp + hh
                            e = epool.tile(
                                [P, 2, QG], bf16, tag="est",
                                name=f"est_{g}_{h}_{kt2}"
                            )
                            cur.ests.setdefault(h, []).append(e)
                            if hh == 1 and kt2 % 2 == 1:
                                # Schraudolph exp on DVE: bf16 bit pattern of
                                # exp(s*INV_SCALE) ~= trunc(A*s + B); offloads
                                # 25% of the exp stream from ScalarE (max rel
                                # err ~3%, partially cancelled by softmax)
                                nc.vector.tensor_scalar(
                                    out=e.bitcast(mybir.dt.int16),
                                    in0=sts[h],
                                    scalar1=SCH_A, scalar2=SCH_B,
                                    op0=ALU.mult, op1=ALU.add,
                                )
                            else:
                                nc.scalar.activation(
                                    out=e, in_=sts[h], func=AF.Exp,
                                    scale=INV_SCALE
                                )

                    # fill phase (pairs 0-2): ACT is starved, so feed it
                    # scores before the heavy projection splices; steady
                    # state: splices first (PE uses the st-ring wait time)
                    if p < 3:
                        emit_scores()
                        emit_splices()
                    else:
                        emit_splices()
                        emit_scores()
                # previous pair's attV is complete; finish it at the start
                # of the next pair (deps met there, no DVE queue parking)
                done_pair = prev_pair
                prev_pair = cur

            # =========== tail: last pair's attV + outproj of group 3 ====
            # per-qt pipelining: as soon as qt's normalize lands, its
            # transposes, outproj chains and output DMA flow while the PE
            # works the next qt.
            finish_pair(done_pair, o_sb_tiles)
            for s in range(NKT // 2):
                attv_slice(prev_pair, s)
            attnT_holder[3] = atpool.tile(
                [P, NFC, QG], bf16, tag="attnT", name="aT_3"
            )
            o_sb3 = o_sb_tiles[3]
            for qt in range(NQT):
                finish_pair(prev_pair, o_sb_tiles, qts=[qt])
                for fc in range(NFC):
                    nc.sync.dma_start_transpose(
                        out=attnT_holder[3][:, fc, ts(qt, P)],
                        in_=o_sb3[:, qt, 2 * fc : 2 * fc + 2, :],
                    )
                for eg in range(2):
                    outproj_chain(attnT_holder[3], 3, qt, eg, copy_act=True)

    nc.compile()
    return nc


def _get_nc(debug=False):
    if "nc" not in _CACHE:
        _CACHE["nc"] = _build()
    return _CACHE["nc"]


def _tf32(a):
    """Round fp32 to the TF32 grid (10-bit mantissa, round-to-nearest-even)."""
    u = np.ascontiguousarray(a, dtype=np.float32).view(np.uint32)
    u = (u + np.uint32(0xFFF) + ((u >> np.uint32(13)) & np.uint32(1))) & np.uint32(
        0xFFFFE000
    )
    return u.view(np.float32)


def _bf16(a):
    import ml_dtypes

    return np.ascontiguousarray(a, dtype=np.float32).astype(ml_dtypes.bfloat16)


def _make_in_maps(inputs):
    q = np.asarray(inputs["query"], dtype=np.float32)
    k = np.asarray(inputs["key"], dtype=np.float32)
    v = np.asarray(inputs["value"], dtype=np.float32)
    wq = np.asarray(inputs["wq"], dtype=np.float32)
    wk = np.asarray(inputs["wk"], dtype=np.float32)
    wv = np.asarray(inputs["wv"], dtype=np.float32)
    wo = np.asarray(inputs["wo"], dtype=np.float32)
    bq = np.asarray(inputs["bq"], dtype=np.float32)
    bk = np.asarray(inputs["bk"], dtype=np.float32)
    bv = np.asarray(inputs["bv"], dtype=np.float32)

    import ml_dtypes

    def _hl(a):
        hi = np.ascontiguousarray(a, dtype=np.float32).astype(
            ml_dtypes.float8_e4m3)
        lo = (a - hi.astype(np.float32)).astype(ml_dtypes.float8_e4m3)
        return hi, lo

    WS = 32.0  # fp8 weight pre-scale (undone via exp scale / ones column)
    xT = [(_hl(q[b].T), _hl(k[b].T), _hl(v[b].T)) for b in range(B)]
    in_maps = []
    for c in range(NCORES):
        b, g = divmod(c, 2)
        sl = slice(g * DC, (g + 1) * DC)
        wq8, wq8l = _hl(wq[:, sl] * WS)
        wk8, wk8l = _hl(wk[:, sl] * WS)
        wv8, wv8l = _hl(wv[:, sl] * WS)
        in_maps.append(
            {
                "xq8": xT[b][0][0], "xq8l": xT[b][0][1],
                "xk8": xT[b][1][0], "xk8l": xT[b][1][1],
                "xv8": xT[b][2][0], "xv8l": xT[b][2][1],
                "wq8": wq8, "wq8l": wq8l,
                "wk8": wk8, "wk8l": wk8l,
                "wv8": wv8, "wv8l": wv8l,
                "wo": _bf16(wo[sl, :]),
                "bq": np.ascontiguousarray(bq[sl] * WS),
                "bk": np.ascontiguousarray(bk[sl] * WS),
                "bv": np.ascontiguousarray(bv[sl] * WS),
            }
        )
    return in_maps


def run(inputs, **kwargs):
    """Run the kernel; returns (full_output, BassKernelResults)."""
    from concourse.bass_utils import run_bass_kernel_spmd

    kwargs.pop("debug", None)
    nc = _get_nc()
    in_maps = _make_in_maps(inputs)
    res = run_bass_kernel_spmd(nc, in_maps, core_ids=list(range(NCORES)), **kwargs)
    bo = np.asarray(inputs["bo"], dtype=np.float32)
    final = np.empty((B, S, D), np.float32)
    for b in range(B):
        final[b] = (
            res.results[2 * b]["out"].astype(np.float32)
            + res.results[2 * b + 1]["out"].astype(np.float32)
            + bo
        )
    return final, res


def kernel(**inputs):
    return run(inputs)[0]
